# revision 22
# baseline (speedup 1.0000x reference)
"""Trainium2 Bass kernel for nn_LiteNTK (graph NTK + ridge solve).

Contract: kernel(**inputs) takes FULL unsharded inputs and returns the FULL
output tuple (pred [128,10], K_SS [64,64]) matching reference.reference.

Math (validated against the jax reference in fp64/fp32/fp16 numpy models):

Per graph (A (64,64), X (64,64)), with A' = A + 1e-4 I:
  u = rowsum(A'), v = colsum(A'), Y = A' X, rn = row-norms^2 of Y
  d1 = sqrt((rn + 1e-4 u^2)/(u v))          # first-layer diag normalizer
  d2 = c*d1 with c^2 = Sn(0.9999) constant  # second-layer diag (S_diag == 1)
  left operand  YtilL^T = (A'/(u*d1) X)^T with aug row 1/d1
  right operand YtilR^T = (A'/(v*d1) X)^T with aug row 1e-4*u/(v*d1)
  p1 = A'^T (1/u), q2 = A'^T (1/v)          # reduction vectors

Pair (i left, j right): one K=65 matmul gives
  S1 = sigma0/(d1_i x d1_j)  directly.
Arc-cosine kernel recursion via half-angle arctan (ACT arctan domain is
[-pi/2, pi/2]; half-angle keeps |arg| <= 1):
  Sc = clip(S1), sq = sqrt(1-Sc^2), at = arctan(Sc/(1+sq)) = arcsin(Sc)/2
  P1h = (at+pi/4)*Sc                        # = pi*DS1*Sc/2
  S2p = 2*P1h + sq (= pi*Sn1), S2c = min(S2p, 0.9999*pi*c^2)
  sq2 = sqrt(1-K^2*S2c^2), at2 = arctan(K*S2c/(1+sq2)), K = 1/(pi*c^2)
  G1h = (at2+pi/4)*P1h, m2h = (at2+pi/4)*S2c
K[i,j] = pd_i . (4/pi^2 G1h) . qd_j + p_i . ((1+c^2)(2K/pi) m2h + sq2/pi) . q_j
         (all constants folded into per-graph weight vectors, /4096 split
          as /64 into each side)
Then KSS_reg = K_SS + 1e-6 tr/64 I, Gauss-Jordan solve (no pivoting; growth
factor 1.0 measured), pred = K_ST^T Z.

Sharding: combined j-axis (64 S + 128 T = 192 graphs) split 24 per core;
each core computes K[:, j_slice] (all 64 i), AllGather of the [24,64]
K-slices, then every core redundantly runs the tiny solve.
"""

import math
import sys

import numpy as np

for _p in ("/opt/trn_rl_repo",):
    if _p not in sys.path:
        sys.path.insert(0, _p)

import concourse.bacc as bacc
import concourse.bass as bass
import concourse.mybir as mybir
from concourse import bass_utils, tile

F32 = mybir.dt.float32
F16 = mybir.dt.float16
BF16 = mybir.dt.bfloat16
AF = mybir.ActivationFunctionType
ALU = mybir.AluOpType

NCORES = 8
N_S, N_T, NN, FEAT = 64, 128, 64, 64
NJ = N_S + N_T          # 192 combined j-graphs
NJ_DEV = NJ // NCORES   # 24 per core
N_SU = N_S // 2         # 32 S-side units (2 graphs each)
N_JU = NJ_DEV // 2      # 12 j-side units
N_IBLK = 8              # i blocks of 8 graphs (free dim 512)
N_JBLK = NJ_DEV // 2    # 12 j blocks of 2 graphs (partition dim 128)

PI = math.pi
A_CLIP = 0.9999
C2 = (A_CLIP * (PI - math.acos(A_CLIP)) + math.sqrt(1.0 - A_CLIP * A_CLIP)) / PI
K_CONST = 1.0 / (PI * C2)
CLIP2 = A_CLIP / K_CONST

# weight-vector constant folds (see module docstring)
QD_SCALE = 4.0 / (PI * PI) / 64.0
WA_SCALE = (1.0 + C2) * (4.0 * K_CONST / PI) / 64.0  # x2: m2h uses S2c' = S2c/2
WB_SCALE = (1.0 / PI) / 64.0
PD_SCALE = 1.0 / 64.0
P_SCALE = 1.0 / 64.0


def _emit_graph_units(nc, sbuf, psum, cols, src_A, src_X, n_units, unit0,
                      ident, epsident, need_left, need_right, dst_L, dst_R):
    """Per-graph precompute for n_units 2-graph units.

    cols: dict of persistent [128, n_cols] fp32 column tiles (one col/unit).
    dst_L/dst_R: [65, 64*2*n_units] operand tiles to fill (rows 0..63).
    """
    for t in range(n_units):
        tc_col = unit0 + t
        abd = sbuf.tile([128, 128], F32, tag="abd")
        nc.vector.memset(abd[:], 0.0)
        nc.sync.dma_start(abd[0:64, 0:64], src_A[2 * t])
        nc.sync.dma_start(abd[64:128, 64:128], src_A[2 * t + 1])
        abde = sbuf.tile([128, 128], F32, tag="abde")
        nc.vector.tensor_tensor(abde[:], abd[:], epsident[:], ALU.add)

        xst = sbuf.tile([128, 64], F32, tag="xst")
        nc.sync.dma_start(xst[0:64, :], src_X[2 * t])
        nc.sync.dma_start(xst[64:128, :], src_X[2 * t + 1])

        nc.vector.tensor_reduce(
            cols["u"][:, tc_col:tc_col + 1], abde[:], mybir.AxisListType.X, ALU.add
        )

        at_ps = psum.tile([128, 128], F32, tag="at_ps")
        nc.tensor.transpose(at_ps[:], abde[:], ident[:])
        abdT = sbuf.tile([128, 128], F32, tag="abdT")
        nc.scalar.copy(abdT[:], at_ps[:])
        nc.vector.tensor_reduce(
            cols["v"][:, tc_col:tc_col + 1], abdT[:], mybir.AxisListType.X, ALU.add
        )

        y_ps = psum.tile([128, 64], F32, tag="y_ps")
        nc.tensor.matmul(y_ps[:], abdT[:], xst[:])
        ysq = sbuf.tile([128, 64], F32, tag="ysq")
        nc.scalar.activation(
            ysq[:], y_ps[:], AF.Square,
            accum_out=cols["rn"][:, tc_col:tc_col + 1],
        )

        # tiny per-unit column math ([128,1] each)
        u = cols["u"][:, tc_col:tc_col + 1]
        v = cols["v"][:, tc_col:tc_col + 1]
        rn = cols["rn"][:, tc_col:tc_col + 1]
        d1 = cols["d1"][:, tc_col:tc_col + 1]
        u14 = sbuf.tile([128, 1], F32, tag="c_u14")
        nc.vector.tensor_scalar(u14[:], u, 1e-4, None, ALU.mult)
        rn2 = sbuf.tile([128, 1], F32, tag="c_rn2")
        nc.vector.scalar_tensor_tensor(rn2[:], u14[:], u, rn, ALU.mult, ALU.add)
        uv = sbuf.tile([128, 1], F32, tag="c_uv")
        nc.vector.tensor_tensor(uv[:], u, v, ALU.mult)
        ruv = sbuf.tile([128, 1], F32, tag="c_ruv")
        nc.vector.reciprocal(ruv[:], uv[:])
        rat = sbuf.tile([128, 1], F32, tag="c_rat")
        nc.vector.tensor_tensor(rat[:], rn2[:], ruv[:], ALU.mult)
        nc.scalar.activation(d1, rat[:], AF.Sqrt)

        ud1 = sbuf.tile([128, 1], F32, tag="c_ud1")
        nc.vector.tensor_tensor(ud1[:], u, d1, ALU.mult)
        sL = sbuf.tile([128, 1], F32, tag="c_sL")
        nc.vector.reciprocal(sL[:], ud1[:])
        vd1 = sbuf.tile([128, 1], F32, tag="c_vd1")
        nc.vector.tensor_tensor(vd1[:], v, d1, ALU.mult)
        sR = sbuf.tile([128, 1], F32, tag="c_sR")
        nc.vector.reciprocal(sR[:], vd1[:])
        ru = sbuf.tile([128, 1], F32, tag="c_ru")
        nc.vector.reciprocal(ru[:], u)
        rv = sbuf.tile([128, 1], F32, tag="c_rv")
        nc.vector.reciprocal(rv[:], v)

        if need_left:
            nc.vector.reciprocal(cols["rd1"][:, tc_col:tc_col + 1], d1)
            p1_ps = psum.tile([128, 1], F32, tag="p1_ps")
            nc.tensor.matmul(p1_ps[:], abde[:], ru[:])
            p1 = sbuf.tile([128, 1], F32, tag="c_p1")
            nc.scalar.copy(p1[:], p1_ps[:])
            nc.vector.tensor_scalar(
                cols["pd"][:, tc_col:tc_col + 1], p1[:], d1, PD_SCALE,
                ALU.mult, ALU.mult,
            )
            nc.vector.tensor_scalar(
                cols["p"][:, tc_col:tc_col + 1], p1[:], P_SCALE, None, ALU.mult
            )
            # left operand: AtilL = A' * rowscale(sL); dst = (AtilL X)^T
            atl = sbuf.tile([128, 128], F32, tag="atl")
            nc.vector.tensor_scalar(atl[:], abde[:], sL[:], None, ALU.mult)
            tl_ps = psum.tile([128, 128], F32, tag="tl_ps")
            nc.tensor.transpose(tl_ps[:], atl[:], ident[:])
            tl_sb = sbuf.tile([128, 128], F32, tag="tl_sb")
            nc.scalar.copy(tl_sb[:], tl_ps[:])
            ytl_ps = psum.tile([64, 128], F32, tag="ytl_ps")
            nc.tensor.matmul(ytl_ps[:], xst[:], tl_sb[:])
            nc.scalar.copy(dst_L[0:64, 128 * t:128 * (t + 1)], ytl_ps[:])

        if need_right:
            jcol = t  # j-local col index within j col tiles
            q2_ps = psum.tile([128, 1], F32, tag="q2_ps")
            nc.tensor.matmul(q2_ps[:], abde[:], rv[:])
            q2 = sbuf.tile([128, 1], F32, tag="c_q2")
            nc.scalar.copy(q2[:], q2_ps[:])
            nc.vector.tensor_scalar(
                cols["qdw"][:, jcol:jcol + 1], q2[:], d1, QD_SCALE,
                ALU.mult, ALU.mult,
            )
            nc.vector.tensor_scalar(
                cols["waw"][:, jcol:jcol + 1], q2[:], WA_SCALE, None, ALU.mult
            )
            nc.vector.tensor_scalar(
                cols["wbw"][:, jcol:jcol + 1], q2[:], WB_SCALE, None, ALU.mult
            )
            # aug_j = 1e-4 * u * sR
            nc.vector.scalar_tensor_tensor(
                cols["augj"][:, jcol:jcol + 1], u, 1e-4, sR[:], ALU.mult, ALU.mult
            )
            atr = sbuf.tile([128, 128], F32, tag="atr")
            nc.vector.tensor_scalar(atr[:], abde[:], sR[:], None, ALU.mult)
            tr_ps = psum.tile([128, 128], F32, tag="tl_ps")
            nc.tensor.transpose(tr_ps[:], atr[:], ident[:])
            tr_sb = sbuf.tile([128, 128], F32, tag="tl_sb")
            nc.scalar.copy(tr_sb[:], tr_ps[:])
            ytr_ps = psum.tile([64, 128], F32, tag="ytl_ps")
            nc.tensor.matmul(ytr_ps[:], xst[:], tr_sb[:])
            nc.scalar.copy(dst_R[0:64, 128 * t:128 * (t + 1)], ytr_ps[:])


def build_program():
    nc = bacc.Bacc("TRN2", target_bir_lowering=False, debug=False,
                   enable_asserts=False, num_devices=NCORES)

    A_S = nc.dram_tensor("A_S", [N_S, NN, NN], F32, kind="ExternalInput").ap()
    X_S = nc.dram_tensor("X_S", [N_S, NN, FEAT], F32, kind="ExternalInput").ap()
    y_S = nc.dram_tensor("y_S", [N_S, 10], F32, kind="ExternalInput").ap()
    A_J = nc.dram_tensor("A_J", [NJ_DEV, NN, NN], F32, kind="ExternalInput").ap()
    X_J = nc.dram_tensor("X_J", [NJ_DEV, NN, FEAT], F32, kind="ExternalInput").ap()
    ident_in = nc.dram_tensor("ident", [128, 128], F32, kind="ExternalInput").ap()
    epsident_in = nc.dram_tensor("epsident", [128, 128], F32, kind="ExternalInput").ap()
    ones_in = nc.dram_tensor("ones_row", [1, 128], F32, kind="ExternalInput").ap()
    negident_in = nc.dram_tensor("negident", [128, 128], F32,
                                 kind="ExternalInput").ap()

    pred_out = nc.dram_tensor("pred", [N_T, 10], F32, kind="ExternalOutput").ap()
    kss_out = nc.dram_tensor("K_SS", [N_S, N_S], F32, kind="ExternalOutput").ap()

    with tile.TileContext(nc) as tc:
        with (
            tc.tile_pool(name="persist", bufs=1) as pp,
            tc.tile_pool(name="dram", bufs=1, space="DRAM") as dram,
        ):
            ident = pp.tile([128, 128], F32)
            epsident = pp.tile([128, 128], F32)
            ones_row = pp.tile([1, 128], F32)
            negident = pp.tile([128, 128], F32)
            nc.sync.dma_start(ident[:], ident_in[:])
            nc.sync.dma_start(epsident[:], epsident_in[:])
            nc.sync.dma_start(ones_row[:], ones_in[:])
            nc.sync.dma_start(negident[:], negident_in[:])

            c025 = pp.tile([128, 1], F32)
            nc.vector.memset(c025[:], 0.25)
            c2 = pp.tile([128, 1], F32)
            nc.vector.memset(c2[:], 2.0)
            RHS_L = pp.tile([65, N_S * NN], F32)      # i-side stacked YtilL^T+aug
            LHS_R = pp.tile([65, NJ_DEV * NN], F32)   # j-side stacked YtilR^T+aug
            # per-jblk expanded weight tiles: slice m is [128, 24] with only
            # columns 2m (partitions 0:64) and 2m+1 (partitions 64:128) nonzero
            QD = pp.tile([128, NJ_DEV * N_JBLK], BF16)
            WA = pp.tile([128, NJ_DEV * N_JBLK], BF16)
            WB = pp.tile([128, NJ_DEV * N_JBLK], BF16)
            K_stage = pp.tile([NJ_DEV, N_S], F32)

            cols = {}
            for name in ("u", "v", "rn", "d1", "rd1", "pd", "p"):
                cols[name] = pp.tile([128, N_SU + N_JU], F32, name=f"col_{name}")
            for name in ("qdw", "waw", "wbw", "augj"):
                cols[name] = pp.tile([128, N_JU], F32, name=f"col_{name}")

            # ---------------- stage A: per-graph precompute ----------------
            with (
                tc.tile_pool(name="pre_sb", bufs=3) as sbuf,
                tc.tile_pool(name="pre_ps", bufs=1, space="PSUM") as psum,
            ):
                _emit_graph_units(nc, sbuf, psum, cols, A_S, X_S, N_SU, 0,
                                  ident, epsident, True, False, RHS_L, None)
                _emit_graph_units(nc, sbuf, psum, cols, A_J, X_J, N_JU, N_SU,
                                  ident, epsident, False, True, None, LHS_R)

            # aug rows: RHS_L[64, 128*t + suba] = rd1cols[suba, t].
            # Engines can't move data across partitions, so transpose the
            # column tiles on the PE first, then DMA the row-major result.
            with tc.tile_pool(name="augt_ps", bufs=1, space="PSUM") as aps:
                rd1T_ps = aps.tile([N_SU, 128], F32, tag="rd1T_ps")
                nc.tensor.transpose(rd1T_ps[:], cols["rd1"][:, 0:N_SU],
                                    ident[:])
                rd1T = pp.tile([N_SU, 128], F32)
                nc.scalar.copy(rd1T[:], rd1T_ps[:])
                augjT_ps = aps.tile([N_JU, 128], F32, tag="augjT_ps")
                nc.tensor.transpose(augjT_ps[:], cols["augj"][:, 0:N_JU],
                                    ident[:])
                augjT = pp.tile([N_JU, 128], F32)
                nc.scalar.copy(augjT[:], augjT_ps[:])
                pdT_ps = aps.tile([N_SU, 128], F32, tag="rd1T_ps")
                nc.tensor.transpose(pdT_ps[:], cols["pd"][:, 0:N_SU], ident[:])
                pdT = pp.tile([N_SU, 128], F32)
                nc.scalar.copy(pdT[:], pdT_ps[:])
                prT_ps = aps.tile([N_SU, 128], F32, tag="rd1T_ps")
                nc.tensor.transpose(prT_ps[:], cols["p"][:, 0:N_SU], ident[:])
                prT = pp.tile([N_SU, 128], F32)
                nc.scalar.copy(prT[:], prT_ps[:])
            nc.sync.dma_start(
                RHS_L[64:65, :].rearrange("p (t s) -> p t s", s=128), rd1T[:]
            )
            nc.sync.dma_start(
                LHS_R[64:65, :].rearrange("p (t s) -> p t s", s=128), augjT[:]
            )
            # weight block-columns (expanded per jblk for psum-accumulating
            # reduction matmuls): even j -> partitions 0:64 col 24m+2m,
            # odd j -> partitions 64:128 col 24m+2m+1
            for w_all, w_col in ((QD, "qdw"), (WA, "waw"), (WB, "wbw")):
                nc.vector.memset(w_all[:], 0.0)
                for m in range(N_JBLK):
                    base = NJ_DEV * m + 2 * m
                    nc.scalar.copy(w_all[0:64, base:base + 1],
                                   cols[w_col][0:64, m:m + 1])
                    nc.scalar.copy(w_all[64:128, base + 1:base + 2],
                                   cols[w_col][64:128, m:m + 1])

            # ---------------- stage B: pair loop ----------------
            # Engine placement tuned from HW microbenches (ns per [128,512]):
            #   DVE: TT 475, TS 269, STT 602, clip-from-psum 599, recip 3212
            #   GPS: TT ~1279, TS(mult/add dual) 680; min on GPS = 7.4us (!)
            #   ACT: 720/op + ~2.7us per table-set switch (phase-batched)
            # bf16 for phase-crossing storage; fp32 for t1/t2 (cancellation
            # near |S|~1) and arctan outputs.
            act_chain = []

            def act(*args, **kw):
                inst = nc.scalar.activation(*args, **kw)
                act_chain.append(inst)
                return inst

            with (
                tc.tile_pool(name="pb_x", bufs=12) as px,     # phase-crossing
                tc.tile_pool(name="pb_t", bufs=2) as pt,      # in-phase temps
                tc.tile_pool(name="pb_ps", bufs=2, space="PSUM") as pps,
                tc.tile_pool(name="pb_mps", bufs=2, space="PSUM") as mps,
            ):
                for iblk in range(N_IBLK):
                    i0 = iblk * 512
                    m1_ps = mps.tile([NJ_DEV, 512], F32, tag="m1")
                    m2_ps = mps.tile([NJ_DEV, 512], F32, tag="m2")
                    Sc_l, sqh_l, x1_l, t1_l = [], [], [], []
                    P1h_l, S2c_l, sq2_l, x2_l, t2_l = [], [], [], [], []

                    # phase 1: matmul + clip + t1/sqh (sqrt set) + recip + x1
                    for jblk in range(N_JBLK):
                        s1_ps = pps.tile([128, 512], F32, tag="s1")
                        nc.tensor.matmul(
                            s1_ps[:],
                            LHS_R[:, jblk * 128:(jblk + 1) * 128],
                            RHS_L[:, i0:i0 + 512],
                        )
                        Sc = px.tile([128, 512], BF16, tag="Sc")
                        nc.vector.tensor_scalar(
                            Sc[:], s1_ps[:], -A_CLIP, A_CLIP, ALU.max, ALU.min
                        )
                        t1 = pt.tile([128, 512], F32, tag="t1")
                        nc.gpsimd.tensor_tensor(t1[:], Sc[:], Sc[:], ALU.mult)
                        sqh = px.tile([128, 512], BF16, tag="sqh")
                        act(sqh[:], t1[:], AF.Sqrt, bias=c025[:], scale=-0.25)
                        d1s = px.tile([128, 512], BF16, tag="d1s")
                        nc.vector.scalar_tensor_tensor(
                            d1s[:], sqh[:], -4.0, t1[:], ALU.mult, ALU.add
                        )
                        Sc_l.append(Sc); sqh_l.append(sqh); t1_l.append(d1s)

                    for jblk in range(N_JBLK):
                        Sc, sqh, d1s = Sc_l[jblk], sqh_l[jblk], t1_l[jblk]
                        # (1+sq)^2 = 2 - (t1 - 4*sqh); affine folded into ACT
                        rp1 = pt.tile([128, 512], BF16, tag="rp1")
                        act(rp1[:], d1s[:], AF.Abs_reciprocal_sqrt,
                            bias=c2[:], scale=-1.0)
                        x1 = px.tile([128, 512], BF16, tag="x1")
                        nc.vector.tensor_tensor(x1[:], Sc[:], rp1[:], ALU.mult)
                        x1_l.append(x1)

                    # phase 2: arctan L1 (trig set) + P1h + S2p/S2c
                    for jblk in range(N_JBLK):
                        Sc, sqh, x1 = Sc_l[jblk], sqh_l[jblk], x1_l[jblk]
                        at1 = pt.tile([128, 512], F32, tag="at1")
                        act(at1[:], x1[:], AF.Arctan)
                        P1h = px.tile([128, 512], BF16, tag="P1h")
                        nc.vector.scalar_tensor_tensor(
                            P1h[:], at1[:], PI / 4.0, Sc[:], ALU.add, ALU.mult
                        )
                        # S2c is half the reference S2 (constants refolded)
                        S2p = pt.tile([128, 512], BF16, tag="S2p")
                        nc.gpsimd.tensor_tensor(S2p[:], P1h[:], sqh[:], ALU.add)
                        S2c = px.tile([128, 512], BF16, tag="S2c")
                        nc.vector.tensor_scalar(S2c[:], S2p[:], 1.0,
                                                CLIP2 / 2.0, ALU.mult, ALU.min)
                        P1h_l.append(P1h); S2c_l.append(S2c)

                    # phase 3: t2/sq2 (sqrt set) + recip + x2
                    for jblk in range(N_JBLK):
                        S2c = S2c_l[jblk]
                        t2 = pt.tile([128, 512], F32, tag="t2")
                        nc.gpsimd.tensor_tensor(t2[:], S2c[:], S2c[:], ALU.mult)
                        sq2 = px.tile([128, 512], BF16, tag="sq2")
                        act(sq2[:], t2[:], AF.Sqrt,
                            bias=1.0, scale=-(4.0 * K_CONST * K_CONST))
                        d2s = px.tile([128, 512], BF16, tag="d2s")
                        nc.vector.scalar_tensor_tensor(
                            d2s[:], sq2[:], -2.0 / (4.0 * K_CONST * K_CONST),
                            t2[:], ALU.mult, ALU.add,
                        )
                        sq2_l.append(sq2); t2_l.append(d2s)

                    for jblk in range(N_JBLK):
                        S2c, sq2, d2s = S2c_l[jblk], sq2_l[jblk], t2_l[jblk]
                        # (1+sq2)^2 = 2 - 4K^2*(t2 - sq2/(2K^2))
                        rp2 = pt.tile([128, 512], BF16, tag="rp2")
                        act(rp2[:], d2s[:], AF.Abs_reciprocal_sqrt,
                            bias=c2[:], scale=-(4.0 * K_CONST * K_CONST))
                        x2 = px.tile([128, 512], BF16, tag="x2")
                        nc.vector.scalar_tensor_tensor(
                            x2[:], S2c[:], 2.0 * K_CONST, rp2[:], ALU.mult,
                            ALU.mult,
                        )
                        x2_l.append(x2)

                    # phase 4: arctan L2 (trig set) + G1h/m2h + reduction MMs
                    for jblk in range(N_JBLK):
                        P1h, S2c, sq2, x2 = (P1h_l[jblk], S2c_l[jblk],
                                             sq2_l[jblk], x2_l[jblk])
                        at2 = pt.tile([128, 512], F32, tag="at2")
                        act(at2[:], x2[:], AF.Arctan)
                        at2p = pt.tile([128, 512], F32, tag="at2p")
                        nc.vector.tensor_scalar(at2p[:], at2[:], PI / 4.0,
                                                None, ALU.add)
                        G1h = pt.tile([128, 512], BF16, tag="G1h")
                        nc.vector.tensor_tensor(G1h[:], at2p[:], P1h[:],
                                                ALU.mult)
                        m2h = pt.tile([128, 512], BF16, tag="m2h")
                        nc.vector.tensor_tensor(m2h[:], at2p[:], S2c[:],
                                                ALU.mult)
                        wslice = slice(NJ_DEV * jblk, NJ_DEV * (jblk + 1))
                        nc.tensor.matmul(
                            m1_ps[:], QD[:, wslice], G1h[:],
                            start=(jblk == 0), stop=(jblk == N_JBLK - 1),
                        )
                        nc.tensor.matmul(
                            m2_ps[:], WA[:, wslice], m2h[:],
                            start=(jblk == 0), stop=False,
                        )
                        nc.tensor.matmul(
                            m2_ps[:], WB[:, wslice], sq2[:],
                            start=False, stop=(jblk == N_JBLK - 1),
                        )

                    # finalize iblk: weighted a-reduction -> K_stage columns
                    pdf = pt.tile([1, 512], F32, tag="pdf")
                    prf = pt.tile([1, 512], F32, tag="prf")
                    nc.sync.dma_start(
                        pdf[:].rearrange("p (t s) -> p t s", s=128),
                        pdT[iblk * 4:(iblk + 1) * 4, :],
                    )
                    nc.sync.dma_start(
                        prf[:].rearrange("p (t s) -> p t s", s=128),
                        prT[iblk * 4:(iblk + 1) * 4, :],
                    )
                    pd_ps = pps.tile([NJ_DEV, 512], F32, tag="pdb_ps")
                    nc.tensor.matmul(pd_ps[:], ones_row[:, 0:NJ_DEV], pdf[:])
                    pdb = pt.tile([NJ_DEV, 512], F32, tag="pdb")
                    nc.scalar.copy(pdb[:], pd_ps[:])
                    pr_ps = pps.tile([NJ_DEV, 512], F32, tag="pdb_ps")
                    nc.tensor.matmul(pr_ps[:], ones_row[:, 0:NJ_DEV], prf[:])
                    prb = pt.tile([NJ_DEV, 512], F32, tag="prb")
                    nc.scalar.copy(prb[:], pr_ps[:])

                    w1 = pt.tile([NJ_DEV, 512], F32, tag="w1")
                    nc.vector.tensor_tensor(w1[:], m1_ps[:], pdb[:], ALU.mult)
                    w2 = pt.tile([NJ_DEV, 512], F32, tag="w2")
                    nc.vector.tensor_tensor(w2[:], m2_ps[:], prb[:], ALU.mult)
                    r1 = pt.tile([NJ_DEV, 8], F32, tag="r1")
                    nc.vector.tensor_reduce(
                        r1[:], w1[:].rearrange("p (j a) -> p j a", j=8),
                        mybir.AxisListType.X, ALU.add,
                    )
                    r2 = pt.tile([NJ_DEV, 8], F32, tag="r2")
                    nc.vector.tensor_reduce(
                        r2[:], w2[:].rearrange("p (j a) -> p j a", j=8),
                        mybir.AxisListType.X, ALU.add,
                    )
                    nc.vector.tensor_tensor(
                        K_stage[:, iblk * 8:(iblk + 1) * 8], r1[:], r2[:], ALU.add
                    )

            # serialize ACT transcendentals in emission order so the scheduler
            # cannot interleave table sets (each switch costs ~2.7us)
            from concourse.tile_rust import add_dep_helper
            for a, b in zip(act_chain[1:], act_chain[:-1]):
                add_dep_helper(a.ins, b.ins, reason="act table-set phase order")

            # ---------------- stage C: all-gather ----------------
            cc_in = dram.tile([NJ_DEV, N_S], F32)
            cc_out = dram.tile([NJ, N_S], F32, addr_space="Shared")
            nc.sync.dma_start(cc_in[:], K_stage[:])
            nc.gpsimd.collective_compute(
                "AllGather",
                ALU.bypass,
                ins=[cc_in[:].opt()],
                outs=[cc_out[:].opt()],
                replica_groups=[list(range(NCORES))],
            )

            # ---------------- stage D: solve + outputs ----------------
            with (
                tc.tile_pool(name="sol_sb", bufs=2) as ss,
                tc.tile_pool(name="sol_ps", bufs=1, space="PSUM") as sps,
            ):
                kssT = ss.tile([64, 64], F32, tag="kssT")
                nc.sync.dma_start(kssT[:], cc_out[0:64, :])
                kstT = ss.tile([128, 64], F32, tag="kstT")
                nc.sync.dma_start(kstT[:], cc_out[64:NJ, :])
                y_sb = ss.tile([64, 10], F32, tag="ysb")
                nc.sync.dma_start(y_sb[:], y_S[:])

                kss_ps = sps.tile([64, 64], F32, tag="kss_ps")
                nc.tensor.transpose(kss_ps[:], kssT[:], ident[0:64, 0:64])
                kss_sb = ss.tile([64, 64], F32, tag="kss_sb")
                nc.scalar.copy(kss_sb[:], kss_ps[:])
                nc.sync.dma_start(kss_out[:], kss_sb[:])

                # lambda = 1e-6 * tr(K_SS)/64, broadcast to [64,1]
                dd = ss.tile([64, 64], F32, tag="dd")
                nc.vector.tensor_tensor(dd[:], kss_sb[:], ident[0:64, 0:64],
                                        ALU.mult)
                dcol = ss.tile([64, 1], F32, tag="dcol")
                nc.vector.tensor_reduce(dcol[:], dd[:], mybir.AxisListType.X,
                                        ALU.add)
                from concourse import bass_isa
                tr_all = ss.tile([64, 1], F32, tag="tr_all")
                nc.gpsimd.partition_all_reduce(tr_all[:], dcol[:], 64,
                                               bass_isa.ReduceOp.add)
                lamcol = ss.tile([64, 1], F32, tag="lamcol")
                nc.vector.tensor_scalar(lamcol[:], tr_all[:], 1e-6 / 64.0,
                                        None, ALU.mult)

                aug = ss.tile([64, 74], F32, tag="aug")
                nc.vector.scalar_tensor_tensor(
                    aug[:, 0:64], ident[0:64, 0:64], lamcol[:], kss_sb[:],
                    ALU.mult, ALU.add,
                )
                nc.scalar.copy(aug[:, 64:74], y_sb[:])

                for k in range(64):
                    # broadcast row k of aug via a zero-stride one-hot lhsT:
                    # out[m,n] = sum_p ident[p,k] * aug[p,n] = aug[k,n]
                    row_ps = sps.tile([64, 74], F32, tag="row_ps")
                    nc.tensor.matmul(
                        row_ps[:],
                        ident[0:64, k:k + 1].broadcast_to([64, 64]),
                        aug[:, :],
                    )
                    rowb = ss.tile([64, 74], F32, tag="rowb")
                    nc.scalar.copy(rowb[:], row_ps[:])
                    rpiv = ss.tile([64, 1], F32, tag="rpiv")
                    nc.vector.reciprocal(rpiv[:], rowb[:, k:k + 1])
                    # nf = -aug[:,k]/piv with nf[k]=0, via negident col
                    # (negident[p,k] = -1 if p!=k else 0)
                    nf = ss.tile([64, 1], F32, tag="nf")
                    nc.vector.scalar_tensor_tensor(
                        nf[:], aug[:, k:k + 1], rpiv[:],
                        negident[0:64, k:k + 1], ALU.mult, ALU.mult,
                    )
                    aug_n = ss.tile([64, 74], F32, tag="aug")
                    nc.vector.scalar_tensor_tensor(
                        aug_n[:], rowb[:], nf[:], aug[:], ALU.mult, ALU.add
                    )
                    aug = aug_n

                dd2 = ss.tile([64, 64], F32, tag="dd")
                nc.vector.tensor_tensor(dd2[:], aug[:, 0:64], ident[0:64, 0:64],
                                        ALU.mult)
                dcol2 = ss.tile([64, 1], F32, tag="dcol")
                nc.vector.tensor_reduce(dcol2[:], dd2[:], mybir.AxisListType.X,
                                        ALU.add)
                rdg = ss.tile([64, 1], F32, tag="rdg")
                nc.vector.reciprocal(rdg[:], dcol2[:])
                Z = ss.tile([64, 10], F32, tag="Z")
                nc.vector.tensor_scalar(Z[:], aug[:, 64:74], rdg[:], None,
                                        ALU.mult)

                kst_ps = sps.tile([64, 128], F32, tag="kst_ps")
                nc.tensor.transpose(kst_ps[:], kstT[:], ident[:])
                kst_sb = ss.tile([64, 128], F32, tag="kst_sb")
                nc.scalar.copy(kst_sb[:], kst_ps[:])
                pred_ps = sps.tile([128, 10], F32, tag="pred_ps")
                nc.tensor.matmul(pred_ps[:], kst_sb[:], Z[:])
                pred_sb = ss.tile([128, 10], F32, tag="pred_sb")
                nc.scalar.copy(pred_sb[:], pred_ps[:])
                nc.sync.dma_start(pred_out[:], pred_sb[:])

    nc.compile()
    return nc


_PROGRAM = None


def _get_program():
    global _PROGRAM
    if _PROGRAM is None:
        _PROGRAM = build_program()
    return _PROGRAM


def make_in_maps(A_S, X_S, y_S, A_T, X_T):
    A_S = np.ascontiguousarray(A_S, dtype=np.float32)
    X_S = np.ascontiguousarray(X_S, dtype=np.float32)
    y_S = np.ascontiguousarray(y_S, dtype=np.float32)
    A_T = np.ascontiguousarray(A_T, dtype=np.float32)
    X_T = np.ascontiguousarray(X_T, dtype=np.float32)
    A_all = np.concatenate([A_S, A_T], axis=0)
    X_all = np.concatenate([X_S, X_T], axis=0)
    ident = np.eye(128, dtype=np.float32)
    epsident = (1e-4 * np.eye(128)).astype(np.float32)
    ones_row = np.ones((1, 128), dtype=np.float32)
    in_maps = []
    for d in range(NCORES):
        in_maps.append({
            "A_S": A_S, "X_S": X_S, "y_S": y_S,
            "A_J": A_all[d * NJ_DEV:(d + 1) * NJ_DEV],
            "X_J": X_all[d * NJ_DEV:(d + 1) * NJ_DEV],
            "ident": ident, "epsident": epsident, "ones_row": ones_row,
            "negident": (np.eye(128, dtype=np.float32) - 1.0),
        })
    return in_maps


def kernel(A_S, X_S, y_S, A_T, X_T):
    nc = _get_program()
    in_maps = make_in_maps(A_S, X_S, y_S, A_T, X_T)
    res = bass_utils.run_bass_kernel_spmd(
        nc, in_maps, core_ids=list(range(NCORES))
    )
    pred = np.asarray(res.results[0]["pred"], dtype=np.float32)
    kss = np.asarray(res.results[0]["K_SS"], dtype=np.float32)
    return pred, kss


# revision 23
# speedup vs baseline: 1.0165x; 1.0165x over previous
"""Trainium2 Bass kernel for nn_LiteNTK (graph NTK + ridge solve).

Contract: kernel(**inputs) takes FULL unsharded inputs and returns the FULL
output tuple (pred [128,10], K_SS [64,64]) matching reference.reference.

Math (validated against the jax reference in fp64/fp32/fp16 numpy models):

Per graph (A (64,64), X (64,64)), with A' = A + 1e-4 I:
  u = rowsum(A'), v = colsum(A'), Y = A' X, rn = row-norms^2 of Y
  d1 = sqrt((rn + 1e-4 u^2)/(u v))          # first-layer diag normalizer
  d2 = c*d1 with c^2 = Sn(0.9999) constant  # second-layer diag (S_diag == 1)
  left operand  YtilL^T = (A'/(u*d1) X)^T with aug row 1/d1
  right operand YtilR^T = (A'/(v*d1) X)^T with aug row 1e-4*u/(v*d1)
  p1 = A'^T (1/u), q2 = A'^T (1/v)          # reduction vectors

Pair (i left, j right): one K=65 matmul gives
  S1 = sigma0/(d1_i x d1_j)  directly.
Arc-cosine kernel recursion via half-angle arctan (ACT arctan domain is
[-pi/2, pi/2]; half-angle keeps |arg| <= 1):
  Sc = clip(S1), sq = sqrt(1-Sc^2), at = arctan(Sc/(1+sq)) = arcsin(Sc)/2
  P1h = (at+pi/4)*Sc                        # = pi*DS1*Sc/2
  S2p = 2*P1h + sq (= pi*Sn1), S2c = min(S2p, 0.9999*pi*c^2)
  sq2 = sqrt(1-K^2*S2c^2), at2 = arctan(K*S2c/(1+sq2)), K = 1/(pi*c^2)
  G1h = (at2+pi/4)*P1h, m2h = (at2+pi/4)*S2c
K[i,j] = pd_i . (4/pi^2 G1h) . qd_j + p_i . ((1+c^2)(2K/pi) m2h + sq2/pi) . q_j
         (all constants folded into per-graph weight vectors, /4096 split
          as /64 into each side)
Then KSS_reg = K_SS + 1e-6 tr/64 I, Gauss-Jordan solve (no pivoting; growth
factor 1.0 measured), pred = K_ST^T Z.

Sharding: combined j-axis (64 S + 128 T = 192 graphs) split 24 per core;
each core computes K[:, j_slice] (all 64 i), AllGather of the [24,64]
K-slices, then every core redundantly runs the tiny solve.
"""

import math
import sys

import numpy as np

for _p in ("/opt/trn_rl_repo",):
    if _p not in sys.path:
        sys.path.insert(0, _p)

import concourse.bacc as bacc
import concourse.bass as bass
import concourse.mybir as mybir
from concourse import bass_utils, tile

F32 = mybir.dt.float32
F16 = mybir.dt.float16
BF16 = mybir.dt.bfloat16
AF = mybir.ActivationFunctionType
ALU = mybir.AluOpType

NCORES = 8
N_S, N_T, NN, FEAT = 64, 128, 64, 64
NJ = N_S + N_T          # 192 combined j-graphs
NJ_DEV = NJ // NCORES   # 24 per core
N_SU = N_S // 2         # 32 S-side units (2 graphs each)
N_JU = NJ_DEV // 2      # 12 j-side units
N_IBLK = 8              # i blocks of 8 graphs (free dim 512)
N_JBLK = NJ_DEV // 2    # 12 j blocks of 2 graphs (partition dim 128)

PI = math.pi
A_CLIP = 0.9999
C2 = (A_CLIP * (PI - math.acos(A_CLIP)) + math.sqrt(1.0 - A_CLIP * A_CLIP)) / PI
K_CONST = 1.0 / (PI * C2)
CLIP2 = A_CLIP / K_CONST

# weight-vector constant folds (see module docstring)
QD_SCALE = 4.0 / (PI * PI) / 64.0
WA_SCALE = (1.0 + C2) * (4.0 * K_CONST / PI) / 64.0  # x2: m2h uses S2c' = S2c/2
WB_SCALE = (1.0 / PI) / 64.0
PD_SCALE = 1.0 / 64.0
P_SCALE = 1.0 / 64.0


def _emit_pass1(nc, sbuf, psum, cols, src_A, src_X, n_units, unit0,
                ident, epsident, cache):
    """Loads, transposes, Y-matmul, u/v/rn reduces. Caches abde/xst tiles."""
    for t in range(n_units):
        tc_col = unit0 + t
        abd = sbuf.tile([128, 128], F32, tag="abd")
        nc.vector.memset(abd[:], 0.0)
        nc.sync.dma_start(abd[0:64, 0:64], src_A[2 * t])
        nc.sync.dma_start(abd[64:128, 64:128], src_A[2 * t + 1])
        abde = sbuf.tile([128, 128], F32, tag="abde", bufs=46)
        nc.vector.tensor_tensor(abde[:], abd[:], epsident[:], ALU.add)

        xst = sbuf.tile([128, 64], F32, tag="xst", bufs=46)
        nc.sync.dma_start(xst[0:64, :], src_X[2 * t])
        nc.sync.dma_start(xst[64:128, :], src_X[2 * t + 1])

        nc.vector.tensor_reduce(
            cols["u"][:, tc_col:tc_col + 1], abde[:], mybir.AxisListType.X,
            ALU.add,
        )
        at_ps = psum.tile([128, 128], F32, tag="at_ps")
        nc.tensor.transpose(at_ps[:], abde[:], ident[:])
        abdT = sbuf.tile([128, 128], F32, tag="abdT")
        nc.scalar.copy(abdT[:], at_ps[:])
        nc.vector.tensor_reduce(
            cols["v"][:, tc_col:tc_col + 1], abdT[:], mybir.AxisListType.X,
            ALU.add,
        )
        y_ps = psum.tile([128, 64], F32, tag="y_ps")
        nc.tensor.matmul(y_ps[:], abdT[:], xst[:])
        ysq = sbuf.tile([128, 64], F32, tag="ysq")
        nc.scalar.activation(
            ysq[:], y_ps[:], AF.Square,
            accum_out=cols["rn"][:, tc_col:tc_col + 1],
        )
        cache.append((abde, xst))


def _emit_col_math(nc, sbuf, cols, n_all):
    """Batched [128, n_all] column math: d1 and all derived scale vectors."""
    A = slice(0, n_all)
    uu = sbuf.tile([128, n_all], F32, tag="b_uu")
    nc.vector.tensor_scalar(uu[:], cols["u"][:, A], 1e-4, None, ALU.mult)
    rn2 = sbuf.tile([128, n_all], F32, tag="b_rn2")
    nc.vector.tensor_tensor(rn2[:], uu[:], cols["u"][:, A], ALU.mult)
    nc.vector.tensor_tensor(rn2[:], rn2[:], cols["rn"][:, A], ALU.add)
    uv = sbuf.tile([128, n_all], F32, tag="b_uv")
    nc.vector.tensor_tensor(uv[:], cols["u"][:, A], cols["v"][:, A], ALU.mult)
    ruv = sbuf.tile([128, n_all], F32, tag="b_ruv")
    nc.vector.reciprocal(ruv[:], uv[:])
    rat = sbuf.tile([128, n_all], F32, tag="b_rat")
    nc.vector.tensor_tensor(rat[:], rn2[:], ruv[:], ALU.mult)
    nc.scalar.activation(cols["d1"][:, A], rat[:], AF.Sqrt)
    ud1 = sbuf.tile([128, n_all], F32, tag="b_ud1")
    nc.vector.tensor_tensor(ud1[:], cols["u"][:, A], cols["d1"][:, A], ALU.mult)
    nc.vector.reciprocal(cols["sL"][:, A], ud1[:])
    vd1 = sbuf.tile([128, n_all], F32, tag="b_vd1")
    nc.vector.tensor_tensor(vd1[:], cols["v"][:, A], cols["d1"][:, A], ALU.mult)
    nc.vector.reciprocal(cols["sR"][:, A], vd1[:])
    nc.vector.reciprocal(cols["ru"][:, A], cols["u"][:, A])
    nc.vector.reciprocal(cols["rv"][:, A], cols["v"][:, A])
    nc.vector.reciprocal(cols["rd1"][:, A], cols["d1"][:, A])


def _emit_pass3(nc, sbuf, psum, cols, n_units, unit0, ident, cache,
                need_left, dst_L, dst_R):
    """Per-unit: p1/q2 matmuls into col tiles + scaled operand build."""
    for t in range(n_units):
        tc_col = unit0 + t
        abde, xst = cache[t]
        if need_left:
            p1_ps = psum.tile([128, 1], F32, tag="p1_ps")
            nc.tensor.matmul(p1_ps[:], abde[:],
                             cols["ru"][:, tc_col:tc_col + 1])
            nc.scalar.copy(cols["p1"][:, tc_col:tc_col + 1], p1_ps[:])
            scale = cols["sL"][:, tc_col:tc_col + 1]
            dst = dst_L
        else:
            q2_ps = psum.tile([128, 1], F32, tag="p1_ps")
            nc.tensor.matmul(q2_ps[:], abde[:],
                             cols["rv"][:, tc_col:tc_col + 1])
            nc.scalar.copy(cols["q2c"][:, t:t + 1], q2_ps[:])
            scale = cols["sR"][:, tc_col:tc_col + 1]
            dst = dst_R
        atl = sbuf.tile([128, 128], F32, tag="atl")
        nc.vector.tensor_scalar(atl[:], abde[:], scale, None, ALU.mult)
        tl_ps = psum.tile([128, 128], F32, tag="tl_ps")
        nc.tensor.transpose(tl_ps[:], atl[:], ident[:])
        tl_sb = sbuf.tile([128, 128], F32, tag="tl_sb")
        nc.scalar.copy(tl_sb[:], tl_ps[:])
        ytl_ps = psum.tile([64, 128], F32, tag="ytl_ps")
        nc.tensor.matmul(ytl_ps[:], xst[:], tl_sb[:])
        nc.scalar.copy(dst[0:64, 128 * t:128 * (t + 1)], ytl_ps[:])


def build_program():
    nc = bacc.Bacc("TRN2", target_bir_lowering=False, debug=False,
                   enable_asserts=False, num_devices=NCORES)

    A_S = nc.dram_tensor("A_S", [N_S, NN, NN], F32, kind="ExternalInput").ap()
    X_S = nc.dram_tensor("X_S", [N_S, NN, FEAT], F32, kind="ExternalInput").ap()
    y_S = nc.dram_tensor("y_S", [N_S, 10], F32, kind="ExternalInput").ap()
    A_J = nc.dram_tensor("A_J", [NJ_DEV, NN, NN], F32, kind="ExternalInput").ap()
    X_J = nc.dram_tensor("X_J", [NJ_DEV, NN, FEAT], F32, kind="ExternalInput").ap()
    ident_in = nc.dram_tensor("ident", [128, 128], F32, kind="ExternalInput").ap()
    epsident_in = nc.dram_tensor("epsident", [128, 128], F32, kind="ExternalInput").ap()
    ones_in = nc.dram_tensor("ones_row", [1, 128], F32, kind="ExternalInput").ap()
    negident_in = nc.dram_tensor("negident", [128, 128], F32,
                                 kind="ExternalInput").ap()

    pred_out = nc.dram_tensor("pred", [N_T, 10], F32, kind="ExternalOutput").ap()
    kss_out = nc.dram_tensor("K_SS", [N_S, N_S], F32, kind="ExternalOutput").ap()

    with tile.TileContext(nc) as tc:
        with (
            tc.tile_pool(name="persist", bufs=1) as pp,
            tc.tile_pool(name="dram", bufs=1, space="DRAM") as dram,
        ):
            ident = pp.tile([128, 128], F32)
            epsident = pp.tile([128, 128], F32)
            ones_row = pp.tile([1, 128], F32)
            negident = pp.tile([128, 128], F32)
            nc.sync.dma_start(ident[:], ident_in[:])
            nc.sync.dma_start(epsident[:], epsident_in[:])
            nc.sync.dma_start(ones_row[:], ones_in[:])
            nc.sync.dma_start(negident[:], negident_in[:])

            c025 = pp.tile([128, 1], F32)
            nc.vector.memset(c025[:], 0.25)
            c2 = pp.tile([128, 1], F32)
            nc.vector.memset(c2[:], 2.0)
            RHS_L = pp.tile([65, N_S * NN], F32)      # i-side stacked YtilL^T+aug
            LHS_R = pp.tile([65, NJ_DEV * NN], F32)   # j-side stacked YtilR^T+aug
            # per-jblk expanded weight tiles: slice m is [128, 24] with only
            # columns 2m (partitions 0:64) and 2m+1 (partitions 64:128) nonzero
            QD = pp.tile([128, NJ_DEV * N_JBLK], BF16)
            WA = pp.tile([128, NJ_DEV * N_JBLK], BF16)
            WB = pp.tile([128, NJ_DEV * N_JBLK], BF16)
            K_stage = pp.tile([NJ_DEV, N_S], F32)

            cols = {}
            for name in ("u", "v", "rn", "d1", "rd1", "pd", "p", "sL", "sR",
                         "ru", "rv", "p1"):
                cols[name] = pp.tile([128, N_SU + N_JU], F32, name=f"col_{name}")
            for name in ("qdw", "waw", "wbw", "augj", "q2c"):
                cols[name] = pp.tile([128, N_JU], F32, name=f"col_{name}")

            # ---------------- stage A: per-graph precompute ----------------
            with (
                tc.tile_pool(name="pre_sb", bufs=3) as sbuf,
                tc.tile_pool(name="pre_ps", bufs=1, space="PSUM") as psum,
            ):
                cache_S, cache_J = [], []
                _emit_pass1(nc, sbuf, psum, cols, A_S, X_S, N_SU, 0,
                            ident, epsident, cache_S)
                _emit_pass1(nc, sbuf, psum, cols, A_J, X_J, N_JU, N_SU,
                            ident, epsident, cache_J)
                _emit_col_math(nc, sbuf, cols, N_SU + N_JU)
                _emit_pass3(nc, sbuf, psum, cols, N_SU, 0, ident, cache_S,
                            True, RHS_L, None)
                _emit_pass3(nc, sbuf, psum, cols, N_JU, N_SU, ident, cache_J,
                            False, None, LHS_R)
                # batched weight-vector math
                SL, JL = slice(0, N_SU), slice(N_SU, N_SU + N_JU)
                nc.vector.scalar_tensor_tensor(
                    cols["pd"][:, SL], cols["p1"][:, SL], PD_SCALE,
                    cols["d1"][:, SL], ALU.mult, ALU.mult,
                )
                nc.vector.tensor_scalar(
                    cols["p"][:, SL], cols["p1"][:, SL], P_SCALE, None, ALU.mult
                )
                nc.vector.scalar_tensor_tensor(
                    cols["qdw"][:, :], cols["q2c"][:, :], QD_SCALE,
                    cols["d1"][:, JL], ALU.mult, ALU.mult,
                )
                nc.vector.tensor_scalar(
                    cols["waw"][:, :], cols["q2c"][:, :], WA_SCALE, None,
                    ALU.mult,
                )
                nc.vector.tensor_scalar(
                    cols["wbw"][:, :], cols["q2c"][:, :], WB_SCALE, None,
                    ALU.mult,
                )
                nc.vector.scalar_tensor_tensor(
                    cols["augj"][:, :], cols["u"][:, JL], 1e-4,
                    cols["sR"][:, JL], ALU.mult, ALU.mult,
                )

            # aug rows: RHS_L[64, 128*t + suba] = rd1cols[suba, t].
            # Engines can't move data across partitions, so transpose the
            # column tiles on the PE first, then DMA the row-major result.
            with tc.tile_pool(name="augt_ps", bufs=1, space="PSUM") as aps:
                rd1T_ps = aps.tile([N_SU, 128], F32, tag="rd1T_ps")
                nc.tensor.transpose(rd1T_ps[:], cols["rd1"][:, 0:N_SU],
                                    ident[:])
                rd1T = pp.tile([N_SU, 128], F32)
                nc.scalar.copy(rd1T[:], rd1T_ps[:])
                augjT_ps = aps.tile([N_JU, 128], F32, tag="augjT_ps")
                nc.tensor.transpose(augjT_ps[:], cols["augj"][:, 0:N_JU],
                                    ident[:])
                augjT = pp.tile([N_JU, 128], F32)
                nc.scalar.copy(augjT[:], augjT_ps[:])
                pdT_ps = aps.tile([N_SU, 128], F32, tag="rd1T_ps")
                nc.tensor.transpose(pdT_ps[:], cols["pd"][:, 0:N_SU], ident[:])
                pdT = pp.tile([N_SU, 128], F32)
                nc.scalar.copy(pdT[:], pdT_ps[:])
                prT_ps = aps.tile([N_SU, 128], F32, tag="rd1T_ps")
                nc.tensor.transpose(prT_ps[:], cols["p"][:, 0:N_SU], ident[:])
                prT = pp.tile([N_SU, 128], F32)
                nc.scalar.copy(prT[:], prT_ps[:])
            nc.sync.dma_start(
                RHS_L[64:65, :].rearrange("p (t s) -> p t s", s=128), rd1T[:]
            )
            nc.sync.dma_start(
                LHS_R[64:65, :].rearrange("p (t s) -> p t s", s=128), augjT[:]
            )
            # weight block-columns (expanded per jblk for psum-accumulating
            # reduction matmuls): even j -> partitions 0:64 col 24m+2m,
            # odd j -> partitions 64:128 col 24m+2m+1
            for w_all, w_col in ((QD, "qdw"), (WA, "waw"), (WB, "wbw")):
                nc.vector.memset(w_all[:], 0.0)
                for m in range(N_JBLK):
                    base = NJ_DEV * m + 2 * m
                    nc.scalar.copy(w_all[0:64, base:base + 1],
                                   cols[w_col][0:64, m:m + 1])
                    nc.scalar.copy(w_all[64:128, base + 1:base + 2],
                                   cols[w_col][64:128, m:m + 1])

            # ---------------- stage B: pair loop ----------------
            # Engine placement tuned from HW microbenches (ns per [128,512]):
            #   DVE: TT 475, TS 269, STT 602, clip-from-psum 599, recip 3212
            #   GPS: TT ~1279, TS(mult/add dual) 680; min on GPS = 7.4us (!)
            #   ACT: 720/op + ~2.7us per table-set switch (phase-batched)
            # bf16 for phase-crossing storage; fp32 for t1/t2 (cancellation
            # near |S|~1) and arctan outputs.
            act_chain = []

            def act(*args, **kw):
                inst = nc.scalar.activation(*args, **kw)
                act_chain.append(inst)
                return inst

            with (
                tc.tile_pool(name="pb_x", bufs=12) as px,     # phase-crossing
                tc.tile_pool(name="pb_t", bufs=2) as pt,      # in-phase temps
                tc.tile_pool(name="pb_ps", bufs=2, space="PSUM") as pps,
                tc.tile_pool(name="pb_mps", bufs=2, space="PSUM") as mps,
            ):
                for iblk in range(N_IBLK):
                    i0 = iblk * 512
                    m1_ps = mps.tile([NJ_DEV, 512], F32, tag="m1")
                    m2_ps = mps.tile([NJ_DEV, 512], F32, tag="m2")
                    Sc_l, sqh_l, x1_l, t1_l = [], [], [], []
                    P1h_l, S2c_l, sq2_l, x2_l, t2_l = [], [], [], [], []

                    # phase 1: matmul + clip + t1/sqh (sqrt set) + recip + x1
                    for jblk in range(N_JBLK):
                        s1_ps = pps.tile([128, 512], F32, tag="s1")
                        nc.tensor.matmul(
                            s1_ps[:],
                            LHS_R[:, jblk * 128:(jblk + 1) * 128],
                            RHS_L[:, i0:i0 + 512],
                        )
                        Sc = px.tile([128, 512], BF16, tag="Sc")
                        nc.vector.tensor_scalar(
                            Sc[:], s1_ps[:], -A_CLIP, A_CLIP, ALU.max, ALU.min
                        )
                        t1 = pt.tile([128, 512], F32, tag="t1")
                        nc.gpsimd.tensor_tensor(t1[:], Sc[:], Sc[:], ALU.mult)
                        sqh = px.tile([128, 512], BF16, tag="sqh")
                        act(sqh[:], t1[:], AF.Sqrt, bias=c025[:], scale=-0.25)
                        d1s = px.tile([128, 512], BF16, tag="d1s")
                        nc.vector.scalar_tensor_tensor(
                            d1s[:], sqh[:], -4.0, t1[:], ALU.mult, ALU.add
                        )
                        Sc_l.append(Sc); sqh_l.append(sqh); t1_l.append(d1s)

                    for jblk in range(N_JBLK):
                        Sc, sqh, d1s = Sc_l[jblk], sqh_l[jblk], t1_l[jblk]
                        # (1+sq)^2 = 2 - (t1 - 4*sqh); affine folded into ACT
                        rp1 = pt.tile([128, 512], BF16, tag="rp1")
                        act(rp1[:], d1s[:], AF.Abs_reciprocal_sqrt,
                            bias=c2[:], scale=-1.0)
                        x1 = px.tile([128, 512], BF16, tag="x1")
                        nc.vector.tensor_tensor(x1[:], Sc[:], rp1[:], ALU.mult)
                        x1_l.append(x1)

                    # phase 2: arctan L1 (trig set) + P1h + S2p/S2c
                    for jblk in range(N_JBLK):
                        Sc, sqh, x1 = Sc_l[jblk], sqh_l[jblk], x1_l[jblk]
                        at1 = pt.tile([128, 512], F32, tag="at1")
                        act(at1[:], x1[:], AF.Arctan)
                        P1h = px.tile([128, 512], BF16, tag="P1h")
                        nc.vector.scalar_tensor_tensor(
                            P1h[:], at1[:], PI / 4.0, Sc[:], ALU.add, ALU.mult
                        )
                        # S2c is half the reference S2 (constants refolded)
                        S2p = pt.tile([128, 512], BF16, tag="S2p")
                        nc.gpsimd.tensor_tensor(S2p[:], P1h[:], sqh[:], ALU.add)
                        S2c = px.tile([128, 512], BF16, tag="S2c")
                        nc.vector.tensor_scalar(S2c[:], S2p[:], 1.0,
                                                CLIP2 / 2.0, ALU.mult, ALU.min)
                        P1h_l.append(P1h); S2c_l.append(S2c)

                    # phase 3: t2/sq2 (sqrt set) + recip + x2
                    for jblk in range(N_JBLK):
                        S2c = S2c_l[jblk]
                        t2 = pt.tile([128, 512], F32, tag="t2")
                        nc.gpsimd.tensor_tensor(t2[:], S2c[:], S2c[:], ALU.mult)
                        sq2 = px.tile([128, 512], BF16, tag="sq2")
                        act(sq2[:], t2[:], AF.Sqrt,
                            bias=1.0, scale=-(4.0 * K_CONST * K_CONST))
                        d2s = px.tile([128, 512], BF16, tag="d2s")
                        nc.vector.scalar_tensor_tensor(
                            d2s[:], sq2[:], -2.0 / (4.0 * K_CONST * K_CONST),
                            t2[:], ALU.mult, ALU.add,
                        )
                        sq2_l.append(sq2); t2_l.append(d2s)

                    for jblk in range(N_JBLK):
                        S2c, sq2, d2s = S2c_l[jblk], sq2_l[jblk], t2_l[jblk]
                        # (1+sq2)^2 = 2 - 4K^2*(t2 - sq2/(2K^2))
                        rp2 = pt.tile([128, 512], BF16, tag="rp2")
                        act(rp2[:], d2s[:], AF.Abs_reciprocal_sqrt,
                            bias=c2[:], scale=-(4.0 * K_CONST * K_CONST))
                        x2 = px.tile([128, 512], BF16, tag="x2")
                        nc.vector.scalar_tensor_tensor(
                            x2[:], S2c[:], 2.0 * K_CONST, rp2[:], ALU.mult,
                            ALU.mult,
                        )
                        x2_l.append(x2)

                    # phase 4: arctan L2 (trig set) + G1h/m2h + reduction MMs
                    for jblk in range(N_JBLK):
                        P1h, S2c, sq2, x2 = (P1h_l[jblk], S2c_l[jblk],
                                             sq2_l[jblk], x2_l[jblk])
                        at2 = pt.tile([128, 512], F32, tag="at2")
                        act(at2[:], x2[:], AF.Arctan)
                        at2p = pt.tile([128, 512], F32, tag="at2p")
                        nc.vector.tensor_scalar(at2p[:], at2[:], PI / 4.0,
                                                None, ALU.add)
                        G1h = pt.tile([128, 512], BF16, tag="G1h")
                        nc.vector.tensor_tensor(G1h[:], at2p[:], P1h[:],
                                                ALU.mult)
                        m2h = pt.tile([128, 512], BF16, tag="m2h")
                        nc.vector.tensor_tensor(m2h[:], at2p[:], S2c[:],
                                                ALU.mult)
                        wslice = slice(NJ_DEV * jblk, NJ_DEV * (jblk + 1))
                        nc.tensor.matmul(
                            m1_ps[:], QD[:, wslice], G1h[:],
                            start=(jblk == 0), stop=(jblk == N_JBLK - 1),
                        )
                        nc.tensor.matmul(
                            m2_ps[:], WA[:, wslice], m2h[:],
                            start=(jblk == 0), stop=False,
                        )
                        nc.tensor.matmul(
                            m2_ps[:], WB[:, wslice], sq2[:],
                            start=False, stop=(jblk == N_JBLK - 1),
                        )

                    # finalize iblk: weighted a-reduction -> K_stage columns
                    pdf = pt.tile([1, 512], F32, tag="pdf")
                    prf = pt.tile([1, 512], F32, tag="prf")
                    nc.sync.dma_start(
                        pdf[:].rearrange("p (t s) -> p t s", s=128),
                        pdT[iblk * 4:(iblk + 1) * 4, :],
                    )
                    nc.sync.dma_start(
                        prf[:].rearrange("p (t s) -> p t s", s=128),
                        prT[iblk * 4:(iblk + 1) * 4, :],
                    )
                    pd_ps = pps.tile([NJ_DEV, 512], F32, tag="pdb_ps")
                    nc.tensor.matmul(pd_ps[:], ones_row[:, 0:NJ_DEV], pdf[:])
                    pdb = pt.tile([NJ_DEV, 512], F32, tag="pdb")
                    nc.scalar.copy(pdb[:], pd_ps[:])
                    pr_ps = pps.tile([NJ_DEV, 512], F32, tag="pdb_ps")
                    nc.tensor.matmul(pr_ps[:], ones_row[:, 0:NJ_DEV], prf[:])
                    prb = pt.tile([NJ_DEV, 512], F32, tag="prb")
                    nc.scalar.copy(prb[:], pr_ps[:])

                    w1 = pt.tile([NJ_DEV, 512], F32, tag="w1")
                    nc.vector.tensor_tensor(w1[:], m1_ps[:], pdb[:], ALU.mult)
                    w2 = pt.tile([NJ_DEV, 512], F32, tag="w2")
                    nc.vector.tensor_tensor(w2[:], m2_ps[:], prb[:], ALU.mult)
                    r1 = pt.tile([NJ_DEV, 8], F32, tag="r1")
                    nc.vector.tensor_reduce(
                        r1[:], w1[:].rearrange("p (j a) -> p j a", j=8),
                        mybir.AxisListType.X, ALU.add,
                    )
                    r2 = pt.tile([NJ_DEV, 8], F32, tag="r2")
                    nc.vector.tensor_reduce(
                        r2[:], w2[:].rearrange("p (j a) -> p j a", j=8),
                        mybir.AxisListType.X, ALU.add,
                    )
                    nc.vector.tensor_tensor(
                        K_stage[:, iblk * 8:(iblk + 1) * 8], r1[:], r2[:], ALU.add
                    )

            # serialize ACT transcendentals in emission order so the scheduler
            # cannot interleave table sets (each switch costs ~2.7us)
            from concourse.tile_rust import add_dep_helper
            for a, b in zip(act_chain[1:], act_chain[:-1]):
                add_dep_helper(a.ins, b.ins, reason="act table-set phase order")

            # ---------------- stage C: all-gather ----------------
            cc_in = dram.tile([NJ_DEV, N_S], F32)
            cc_out = dram.tile([NJ, N_S], F32, addr_space="Shared")
            nc.sync.dma_start(cc_in[:], K_stage[:])
            nc.gpsimd.collective_compute(
                "AllGather",
                ALU.bypass,
                ins=[cc_in[:].opt()],
                outs=[cc_out[:].opt()],
                replica_groups=[list(range(NCORES))],
            )

            # ---------------- stage D: solve + outputs ----------------
            with (
                tc.tile_pool(name="sol_sb", bufs=2) as ss,
                tc.tile_pool(name="sol_ps", bufs=1, space="PSUM") as sps,
            ):
                kssT = ss.tile([64, 64], F32, tag="kssT")
                nc.sync.dma_start(kssT[:], cc_out[0:64, :])
                kstT = ss.tile([128, 64], F32, tag="kstT")
                nc.sync.dma_start(kstT[:], cc_out[64:NJ, :])
                y_sb = ss.tile([64, 10], F32, tag="ysb")
                nc.sync.dma_start(y_sb[:], y_S[:])

                kss_ps = sps.tile([64, 64], F32, tag="kss_ps")
                nc.tensor.transpose(kss_ps[:], kssT[:], ident[0:64, 0:64])
                kss_sb = ss.tile([64, 64], F32, tag="kss_sb")
                nc.scalar.copy(kss_sb[:], kss_ps[:])
                nc.sync.dma_start(kss_out[:], kss_sb[:])

                # lambda = 1e-6 * tr(K_SS)/64, broadcast to [64,1]
                dd = ss.tile([64, 64], F32, tag="dd")
                nc.vector.tensor_tensor(dd[:], kss_sb[:], ident[0:64, 0:64],
                                        ALU.mult)
                dcol = ss.tile([64, 1], F32, tag="dcol")
                nc.vector.tensor_reduce(dcol[:], dd[:], mybir.AxisListType.X,
                                        ALU.add)
                from concourse import bass_isa
                tr_all = ss.tile([64, 1], F32, tag="tr_all")
                nc.gpsimd.partition_all_reduce(tr_all[:], dcol[:], 64,
                                               bass_isa.ReduceOp.add)
                lamcol = ss.tile([64, 1], F32, tag="lamcol")
                nc.vector.tensor_scalar(lamcol[:], tr_all[:], 1e-6 / 64.0,
                                        None, ALU.mult)

                aug = ss.tile([64, 74], F32, tag="aug")
                nc.vector.scalar_tensor_tensor(
                    aug[:, 0:64], ident[0:64, 0:64], lamcol[:], kss_sb[:],
                    ALU.mult, ALU.add,
                )
                nc.scalar.copy(aug[:, 64:74], y_sb[:])

                for k in range(64):
                    # broadcast row k of aug via a zero-stride one-hot lhsT:
                    # out[m,n] = sum_p ident[p,k] * aug[p,n] = aug[k,n]
                    row_ps = sps.tile([64, 74], F32, tag="row_ps")
                    nc.tensor.matmul(
                        row_ps[:],
                        ident[0:64, k:k + 1].broadcast_to([64, 64]),
                        aug[:, :],
                    )
                    rowb = ss.tile([64, 74], F32, tag="rowb")
                    nc.scalar.copy(rowb[:], row_ps[:])
                    rpiv = ss.tile([64, 1], F32, tag="rpiv")
                    nc.vector.reciprocal(rpiv[:], rowb[:, k:k + 1])
                    # nf = -aug[:,k]/piv with nf[k]=0, via negident col
                    # (negident[p,k] = -1 if p!=k else 0)
                    nf = ss.tile([64, 1], F32, tag="nf")
                    nc.vector.scalar_tensor_tensor(
                        nf[:], aug[:, k:k + 1], rpiv[:],
                        negident[0:64, k:k + 1], ALU.mult, ALU.mult,
                    )
                    aug_n = ss.tile([64, 74], F32, tag="aug")
                    nc.vector.scalar_tensor_tensor(
                        aug_n[:], rowb[:], nf[:], aug[:], ALU.mult, ALU.add
                    )
                    aug = aug_n

                dd2 = ss.tile([64, 64], F32, tag="dd")
                nc.vector.tensor_tensor(dd2[:], aug[:, 0:64], ident[0:64, 0:64],
                                        ALU.mult)
                dcol2 = ss.tile([64, 1], F32, tag="dcol")
                nc.vector.tensor_reduce(dcol2[:], dd2[:], mybir.AxisListType.X,
                                        ALU.add)
                rdg = ss.tile([64, 1], F32, tag="rdg")
                nc.vector.reciprocal(rdg[:], dcol2[:])
                Z = ss.tile([64, 10], F32, tag="Z")
                nc.vector.tensor_scalar(Z[:], aug[:, 64:74], rdg[:], None,
                                        ALU.mult)

                kst_ps = sps.tile([64, 128], F32, tag="kst_ps")
                nc.tensor.transpose(kst_ps[:], kstT[:], ident[:])
                kst_sb = ss.tile([64, 128], F32, tag="kst_sb")
                nc.scalar.copy(kst_sb[:], kst_ps[:])
                pred_ps = sps.tile([128, 10], F32, tag="pred_ps")
                nc.tensor.matmul(pred_ps[:], kst_sb[:], Z[:])
                pred_sb = ss.tile([128, 10], F32, tag="pred_sb")
                nc.scalar.copy(pred_sb[:], pred_ps[:])
                nc.sync.dma_start(pred_out[:], pred_sb[:])

    nc.compile()
    return nc


_PROGRAM = None


def _get_program():
    global _PROGRAM
    if _PROGRAM is None:
        _PROGRAM = build_program()
    return _PROGRAM


def make_in_maps(A_S, X_S, y_S, A_T, X_T):
    A_S = np.ascontiguousarray(A_S, dtype=np.float32)
    X_S = np.ascontiguousarray(X_S, dtype=np.float32)
    y_S = np.ascontiguousarray(y_S, dtype=np.float32)
    A_T = np.ascontiguousarray(A_T, dtype=np.float32)
    X_T = np.ascontiguousarray(X_T, dtype=np.float32)
    A_all = np.concatenate([A_S, A_T], axis=0)
    X_all = np.concatenate([X_S, X_T], axis=0)
    ident = np.eye(128, dtype=np.float32)
    epsident = (1e-4 * np.eye(128)).astype(np.float32)
    ones_row = np.ones((1, 128), dtype=np.float32)
    in_maps = []
    for d in range(NCORES):
        in_maps.append({
            "A_S": A_S, "X_S": X_S, "y_S": y_S,
            "A_J": A_all[d * NJ_DEV:(d + 1) * NJ_DEV],
            "X_J": X_all[d * NJ_DEV:(d + 1) * NJ_DEV],
            "ident": ident, "epsident": epsident, "ones_row": ones_row,
            "negident": (np.eye(128, dtype=np.float32) - 1.0),
        })
    return in_maps


def kernel(A_S, X_S, y_S, A_T, X_T):
    nc = _get_program()
    in_maps = make_in_maps(A_S, X_S, y_S, A_T, X_T)
    res = bass_utils.run_bass_kernel_spmd(
        nc, in_maps, core_ids=list(range(NCORES))
    )
    pred = np.asarray(res.results[0]["pred"], dtype=np.float32)
    kss = np.asarray(res.results[0]["K_SS"], dtype=np.float32)
    return pred, kss


# revision 24
# speedup vs baseline: 1.0366x; 1.0199x over previous
"""Trainium2 Bass kernel for nn_LiteNTK (graph NTK + ridge solve).

Contract: kernel(**inputs) takes FULL unsharded inputs and returns the FULL
output tuple (pred [128,10], K_SS [64,64]) matching reference.reference.

Math (validated against the jax reference in fp64/fp32/fp16 numpy models):

Per graph (A (64,64), X (64,64)), with A' = A + 1e-4 I:
  u = rowsum(A'), v = colsum(A'), Y = A' X, rn = row-norms^2 of Y
  d1 = sqrt((rn + 1e-4 u^2)/(u v))          # first-layer diag normalizer
  d2 = c*d1 with c^2 = Sn(0.9999) constant  # second-layer diag (S_diag == 1)
  left operand  YtilL^T = (A'/(u*d1) X)^T with aug row 1/d1
  right operand YtilR^T = (A'/(v*d1) X)^T with aug row 1e-4*u/(v*d1)
  p1 = A'^T (1/u), q2 = A'^T (1/v)          # reduction vectors

Pair (i left, j right): one K=65 matmul gives
  S1 = sigma0/(d1_i x d1_j)  directly.
Arc-cosine kernel recursion via half-angle arctan (ACT arctan domain is
[-pi/2, pi/2]; half-angle keeps |arg| <= 1):
  Sc = clip(S1), sq = sqrt(1-Sc^2), at = arctan(Sc/(1+sq)) = arcsin(Sc)/2
  P1h = (at+pi/4)*Sc                        # = pi*DS1*Sc/2
  S2p = 2*P1h + sq (= pi*Sn1), S2c = min(S2p, 0.9999*pi*c^2)
  sq2 = sqrt(1-K^2*S2c^2), at2 = arctan(K*S2c/(1+sq2)), K = 1/(pi*c^2)
  G1h = (at2+pi/4)*P1h, m2h = (at2+pi/4)*S2c
K[i,j] = pd_i . (4/pi^2 G1h) . qd_j + p_i . ((1+c^2)(2K/pi) m2h + sq2/pi) . q_j
         (all constants folded into per-graph weight vectors, /4096 split
          as /64 into each side)
Then KSS_reg = K_SS + 1e-6 tr/64 I, Gauss-Jordan solve (no pivoting; growth
factor 1.0 measured), pred = K_ST^T Z.

Sharding: combined j-axis (64 S + 128 T = 192 graphs) split 24 per core;
each core computes K[:, j_slice] (all 64 i), AllGather of the [24,64]
K-slices, then every core redundantly runs the tiny solve.
"""

import math
import sys

import numpy as np

for _p in ("/opt/trn_rl_repo",):
    if _p not in sys.path:
        sys.path.insert(0, _p)

import concourse.bacc as bacc
import concourse.bass as bass
import concourse.mybir as mybir
from concourse import bass_utils, tile

F32 = mybir.dt.float32
F16 = mybir.dt.float16
BF16 = mybir.dt.bfloat16
AF = mybir.ActivationFunctionType
ALU = mybir.AluOpType

NCORES = 8
N_S, N_T, NN, FEAT = 64, 128, 64, 64
NJ = N_S + N_T          # 192 combined j-graphs
NJ_DEV = NJ // NCORES   # 24 per core
N_SU = N_S // 2         # 32 S-side units (2 graphs each)
N_JU = NJ_DEV // 2      # 12 j-side units
N_IBLK = 8              # i blocks of 8 graphs (free dim 512)
N_JBLK = NJ_DEV // 2    # 12 j blocks of 2 graphs (partition dim 128)

PI = math.pi
A_CLIP = 0.9999
C2 = (A_CLIP * (PI - math.acos(A_CLIP)) + math.sqrt(1.0 - A_CLIP * A_CLIP)) / PI
K_CONST = 1.0 / (PI * C2)
CLIP2 = A_CLIP / K_CONST

# weight-vector constant folds (see module docstring)
QD_SCALE = 4.0 / (PI * PI) / 64.0
WA_SCALE = (1.0 + C2) * (4.0 * K_CONST / PI) / 64.0  # x2: m2h uses S2c' = S2c/2
WB_SCALE = (1.0 / PI) / 64.0
PD_SCALE = 1.0 / 64.0
P_SCALE = 1.0 / 64.0


def _emit_pass1(nc, sbuf, psum, cols, src_A, src_X, n_units, unit0,
                ident, epsident, cache):
    """Loads, transposes, Y-matmul, u/v/rn reduces. Caches abde/xst tiles."""
    for t in range(n_units):
        tc_col = unit0 + t
        abd = sbuf.tile([128, 128], F32, tag="abd")
        nc.vector.memset(abd[:], 0.0)
        nc.sync.dma_start(abd[0:64, 0:64], src_A[2 * t])
        nc.sync.dma_start(abd[64:128, 64:128], src_A[2 * t + 1])
        abde = sbuf.tile([128, 128], F32, tag="abde", bufs=46)
        nc.vector.tensor_tensor(abde[:], abd[:], epsident[:], ALU.add)

        xst = sbuf.tile([128, 64], F32, tag="xst", bufs=46)
        nc.sync.dma_start(xst[0:64, :], src_X[2 * t])
        nc.sync.dma_start(xst[64:128, :], src_X[2 * t + 1])

        nc.vector.tensor_reduce(
            cols["u"][:, tc_col:tc_col + 1], abde[:], mybir.AxisListType.X,
            ALU.add,
        )
        at_ps = psum.tile([128, 128], F32, tag="at_ps")
        nc.tensor.transpose(at_ps[:], abde[:], ident[:])
        abdT = sbuf.tile([128, 128], F32, tag="abdT")
        nc.scalar.copy(abdT[:], at_ps[:])
        nc.vector.tensor_reduce(
            cols["v"][:, tc_col:tc_col + 1], abdT[:], mybir.AxisListType.X,
            ALU.add,
        )
        y_ps = psum.tile([128, 64], F32, tag="y_ps")
        nc.tensor.matmul(y_ps[:], abdT[:], xst[:])
        ysq = sbuf.tile([128, 64], F32, tag="ysq")
        nc.scalar.activation(
            ysq[:], y_ps[:], AF.Square,
            accum_out=cols["rn"][:, tc_col:tc_col + 1],
        )
        cache.append((abde, xst))


def _emit_col_math(nc, sbuf, cols, n_all):
    """Batched [128, n_all] column math: d1 and all derived scale vectors."""
    A = slice(0, n_all)
    uu = sbuf.tile([128, n_all], F32, tag="b_uu")
    nc.vector.tensor_scalar(uu[:], cols["u"][:, A], 1e-4, None, ALU.mult)
    rn2 = sbuf.tile([128, n_all], F32, tag="b_rn2")
    nc.vector.tensor_tensor(rn2[:], uu[:], cols["u"][:, A], ALU.mult)
    nc.vector.tensor_tensor(rn2[:], rn2[:], cols["rn"][:, A], ALU.add)
    uv = sbuf.tile([128, n_all], F32, tag="b_uv")
    nc.vector.tensor_tensor(uv[:], cols["u"][:, A], cols["v"][:, A], ALU.mult)
    ruv = sbuf.tile([128, n_all], F32, tag="b_ruv")
    nc.vector.reciprocal(ruv[:], uv[:])
    rat = sbuf.tile([128, n_all], F32, tag="b_rat")
    nc.vector.tensor_tensor(rat[:], rn2[:], ruv[:], ALU.mult)
    nc.scalar.activation(cols["d1"][:, A], rat[:], AF.Sqrt)
    ud1 = sbuf.tile([128, n_all], F32, tag="b_ud1")
    nc.vector.tensor_tensor(ud1[:], cols["u"][:, A], cols["d1"][:, A], ALU.mult)
    nc.vector.reciprocal(cols["sL"][:, A], ud1[:])
    vd1 = sbuf.tile([128, n_all], F32, tag="b_vd1")
    nc.vector.tensor_tensor(vd1[:], cols["v"][:, A], cols["d1"][:, A], ALU.mult)
    nc.vector.reciprocal(cols["sR"][:, A], vd1[:])
    nc.vector.reciprocal(cols["ru"][:, A], cols["u"][:, A])
    nc.vector.reciprocal(cols["rv"][:, A], cols["v"][:, A])
    nc.vector.reciprocal(cols["rd1"][:, A], cols["d1"][:, A])


def _emit_pass3(nc, sbuf, psum, cols, n_units, unit0, ident, cache,
                need_left, dst_L, dst_R):
    """Per-unit: p1/q2 matmuls into col tiles + scaled operand build."""
    for t in range(n_units):
        tc_col = unit0 + t
        abde, xst = cache[t]
        if need_left:
            p1_ps = psum.tile([128, 1], F32, tag="p1_ps")
            nc.tensor.matmul(p1_ps[:], abde[:],
                             cols["ru"][:, tc_col:tc_col + 1])
            nc.scalar.copy(cols["p1"][:, tc_col:tc_col + 1], p1_ps[:])
            scale = cols["sL"][:, tc_col:tc_col + 1]
            dst = dst_L
        else:
            q2_ps = psum.tile([128, 1], F32, tag="p1_ps")
            nc.tensor.matmul(q2_ps[:], abde[:],
                             cols["rv"][:, tc_col:tc_col + 1])
            nc.scalar.copy(cols["q2c"][:, t:t + 1], q2_ps[:])
            scale = cols["sR"][:, tc_col:tc_col + 1]
            dst = dst_R
        atl = sbuf.tile([128, 128], F32, tag="atl")
        nc.vector.tensor_scalar(atl[:], abde[:], scale, None, ALU.mult)
        tl_ps = psum.tile([128, 128], F32, tag="tl_ps")
        nc.tensor.transpose(tl_ps[:], atl[:], ident[:])
        tl_sb = sbuf.tile([128, 128], F32, tag="tl_sb")
        nc.scalar.copy(tl_sb[:], tl_ps[:])
        ytl_ps = psum.tile([64, 128], F32, tag="ytl_ps")
        nc.tensor.matmul(ytl_ps[:], xst[:], tl_sb[:])
        nc.scalar.copy(dst[0:64, 128 * t:128 * (t + 1)], ytl_ps[:])


def build_program():
    nc = bacc.Bacc("TRN2", target_bir_lowering=False, debug=False,
                   enable_asserts=False, num_devices=NCORES)

    A_S = nc.dram_tensor("A_S", [N_S, NN, NN], F32, kind="ExternalInput").ap()
    X_S = nc.dram_tensor("X_S", [N_S, NN, FEAT], F32, kind="ExternalInput").ap()
    y_S = nc.dram_tensor("y_S", [N_S, 10], F32, kind="ExternalInput").ap()
    A_J = nc.dram_tensor("A_J", [NJ_DEV, NN, NN], F32, kind="ExternalInput").ap()
    X_J = nc.dram_tensor("X_J", [NJ_DEV, NN, FEAT], F32, kind="ExternalInput").ap()
    ident_in = nc.dram_tensor("ident", [128, 128], F32, kind="ExternalInput").ap()
    epsident_in = nc.dram_tensor("epsident", [128, 128], F32, kind="ExternalInput").ap()
    ones_in = nc.dram_tensor("ones_row", [1, 128], F32, kind="ExternalInput").ap()
    negident_in = nc.dram_tensor("negident", [128, 128], F32,
                                 kind="ExternalInput").ap()

    pred_out = nc.dram_tensor("pred", [N_T, 10], F32, kind="ExternalOutput").ap()
    kss_out = nc.dram_tensor("K_SS", [N_S, N_S], F32, kind="ExternalOutput").ap()

    with tile.TileContext(nc) as tc:
        with (
            tc.tile_pool(name="persist", bufs=1) as pp,
            tc.tile_pool(name="dram", bufs=1, space="DRAM") as dram,
        ):
            ident = pp.tile([128, 128], F32)
            epsident = pp.tile([128, 128], F32)
            ones_row = pp.tile([1, 128], F32)
            negident = pp.tile([128, 128], F32)
            nc.sync.dma_start(ident[:], ident_in[:])
            nc.sync.dma_start(epsident[:], epsident_in[:])
            nc.sync.dma_start(ones_row[:], ones_in[:])
            nc.sync.dma_start(negident[:], negident_in[:])

            c025 = pp.tile([128, 1], F32)
            nc.vector.memset(c025[:], 0.25)
            c2 = pp.tile([128, 1], F32)
            nc.vector.memset(c2[:], 2.0)
            RHS_L = pp.tile([65, N_S * NN], F32)      # i-side stacked YtilL^T+aug
            LHS_R = pp.tile([65, NJ_DEV * NN], F32)   # j-side stacked YtilR^T+aug
            # per-jblk expanded weight tiles: slice m is [128, 24] with only
            # columns 2m (partitions 0:64) and 2m+1 (partitions 64:128) nonzero
            QD = pp.tile([128, NJ_DEV * N_JBLK], BF16)
            WA = pp.tile([128, NJ_DEV * N_JBLK], BF16)
            WB = pp.tile([128, NJ_DEV * N_JBLK], BF16)
            K_stage = pp.tile([NJ_DEV, N_S], F32)

            cols = {}
            for name in ("u", "v", "rn", "d1", "rd1", "pd", "p", "sL", "sR",
                         "ru", "rv", "p1"):
                cols[name] = pp.tile([128, N_SU + N_JU], F32, name=f"col_{name}")
            for name in ("qdw", "waw", "wbw", "augj", "q2c"):
                cols[name] = pp.tile([128, N_JU], F32, name=f"col_{name}")

            # ---------------- stage A: per-graph precompute ----------------
            with (
                tc.tile_pool(name="pre_sb", bufs=3) as sbuf,
                tc.tile_pool(name="pre_ps", bufs=1, space="PSUM") as psum,
            ):
                cache_S, cache_J = [], []
                _emit_pass1(nc, sbuf, psum, cols, A_S, X_S, N_SU, 0,
                            ident, epsident, cache_S)
                _emit_pass1(nc, sbuf, psum, cols, A_J, X_J, N_JU, N_SU,
                            ident, epsident, cache_J)
                _emit_col_math(nc, sbuf, cols, N_SU + N_JU)
                _emit_pass3(nc, sbuf, psum, cols, N_SU, 0, ident, cache_S,
                            True, RHS_L, None)
                _emit_pass3(nc, sbuf, psum, cols, N_JU, N_SU, ident, cache_J,
                            False, None, LHS_R)
                # batched weight-vector math
                SL, JL = slice(0, N_SU), slice(N_SU, N_SU + N_JU)
                nc.vector.scalar_tensor_tensor(
                    cols["pd"][:, SL], cols["p1"][:, SL], PD_SCALE,
                    cols["d1"][:, SL], ALU.mult, ALU.mult,
                )
                nc.vector.tensor_scalar(
                    cols["p"][:, SL], cols["p1"][:, SL], P_SCALE, None, ALU.mult
                )
                nc.vector.scalar_tensor_tensor(
                    cols["qdw"][:, :], cols["q2c"][:, :], QD_SCALE,
                    cols["d1"][:, JL], ALU.mult, ALU.mult,
                )
                nc.vector.tensor_scalar(
                    cols["waw"][:, :], cols["q2c"][:, :], WA_SCALE, None,
                    ALU.mult,
                )
                nc.vector.tensor_scalar(
                    cols["wbw"][:, :], cols["q2c"][:, :], WB_SCALE, None,
                    ALU.mult,
                )
                nc.vector.scalar_tensor_tensor(
                    cols["augj"][:, :], cols["u"][:, JL], 1e-4,
                    cols["sR"][:, JL], ALU.mult, ALU.mult,
                )

            # aug rows: RHS_L[64, 128*t + suba] = rd1cols[suba, t].
            # Engines can't move data across partitions, so transpose the
            # column tiles on the PE first, then DMA the row-major result.
            with tc.tile_pool(name="augt_ps", bufs=1, space="PSUM") as aps:
                rd1T_ps = aps.tile([N_SU, 128], F32, tag="rd1T_ps")
                nc.tensor.transpose(rd1T_ps[:], cols["rd1"][:, 0:N_SU],
                                    ident[:])
                rd1T = pp.tile([N_SU, 128], F32)
                nc.scalar.copy(rd1T[:], rd1T_ps[:])
                augjT_ps = aps.tile([N_JU, 128], F32, tag="augjT_ps")
                nc.tensor.transpose(augjT_ps[:], cols["augj"][:, 0:N_JU],
                                    ident[:])
                augjT = pp.tile([N_JU, 128], F32)
                nc.scalar.copy(augjT[:], augjT_ps[:])
                pdT_ps = aps.tile([N_SU, 128], F32, tag="rd1T_ps")
                nc.tensor.transpose(pdT_ps[:], cols["pd"][:, 0:N_SU], ident[:])
                pdT = pp.tile([N_SU, 128], F32)
                nc.scalar.copy(pdT[:], pdT_ps[:])
                prT_ps = aps.tile([N_SU, 128], F32, tag="rd1T_ps")
                nc.tensor.transpose(prT_ps[:], cols["p"][:, 0:N_SU], ident[:])
                prT = pp.tile([N_SU, 128], F32)
                nc.scalar.copy(prT[:], prT_ps[:])
            nc.sync.dma_start(
                RHS_L[64:65, :].rearrange("p (t s) -> p t s", s=128), rd1T[:]
            )
            nc.sync.dma_start(
                LHS_R[64:65, :].rearrange("p (t s) -> p t s", s=128), augjT[:]
            )
            # weight block-columns (expanded per jblk for psum-accumulating
            # reduction matmuls): even j -> partitions 0:64 col 24m+2m,
            # odd j -> partitions 64:128 col 24m+2m+1
            for w_all, w_col in ((QD, "qdw"), (WA, "waw"), (WB, "wbw")):
                nc.vector.memset(w_all[:], 0.0)
                for m in range(N_JBLK):
                    base = NJ_DEV * m + 2 * m
                    nc.scalar.copy(w_all[0:64, base:base + 1],
                                   cols[w_col][0:64, m:m + 1])
                    nc.scalar.copy(w_all[64:128, base + 1:base + 2],
                                   cols[w_col][64:128, m:m + 1])

            # ---------------- stage B: pair loop ----------------
            # Engine placement tuned from HW microbenches (ns per [128,512]):
            #   DVE: TT 475, TS 269, STT 602, clip-from-psum 599, recip 3212
            #   GPS: TT ~1279, TS(mult/add dual) 680; min on GPS = 7.4us (!)
            #   ACT: 720/op + ~2.7us per table-set switch (phase-batched)
            # bf16 for phase-crossing storage; fp32 for t1/t2 (cancellation
            # near |S|~1) and arctan outputs.
            act_chain = []

            def act(*args, **kw):
                inst = nc.scalar.activation(*args, **kw)
                act_chain.append(inst)
                return inst

            with (
                tc.tile_pool(name="pb_x", bufs=13) as px,     # phase-crossing
                tc.tile_pool(name="pb_t", bufs=2) as pt,      # in-phase temps
                tc.tile_pool(name="pb_ps", bufs=2, space="PSUM") as pps,
                tc.tile_pool(name="pb_mps", bufs=2, space="PSUM") as mps,
            ):
                for iblk in range(N_IBLK):
                    i0 = iblk * 512
                    m1_ps = mps.tile([NJ_DEV, 512], F32, tag="m1")
                    m2_ps = mps.tile([NJ_DEV, 512], F32, tag="m2")
                    Sc_l, sqh_l, x1_l, t1_l = [], [], [], []
                    P1h_l, S2c_l, sq2_l, x2_l, t2_l = [], [], [], [], []

                    # phase 1: matmul + clip + t1/sqh (sqrt set) + recip + x1
                    for jblk in range(N_JBLK):
                        s1_ps = pps.tile([128, 512], F32, tag="s1")
                        nc.tensor.matmul(
                            s1_ps[:],
                            LHS_R[:, jblk * 128:(jblk + 1) * 128],
                            RHS_L[:, i0:i0 + 512],
                        )
                        Sc = px.tile([128, 512], BF16, tag="Sc")
                        nc.vector.tensor_scalar(
                            Sc[:], s1_ps[:], -A_CLIP, A_CLIP, ALU.max, ALU.min
                        )
                        t1 = pt.tile([128, 512], F32, tag="t1")
                        nc.gpsimd.tensor_tensor(t1[:], Sc[:], Sc[:], ALU.mult)
                        sqh = px.tile([128, 512], BF16, tag="sqh")
                        act(sqh[:], t1[:], AF.Sqrt, bias=c025[:], scale=-0.25)
                        d1s = px.tile([128, 512], BF16, tag="d1s")
                        nc.vector.scalar_tensor_tensor(
                            d1s[:], sqh[:], -4.0, t1[:], ALU.mult, ALU.add
                        )
                        Sc_l.append(Sc); sqh_l.append(sqh); t1_l.append(d1s)

                    for jblk in range(N_JBLK):
                        Sc, sqh, d1s = Sc_l[jblk], sqh_l[jblk], t1_l[jblk]
                        # (1+sq)^2 = 2 - (t1 - 4*sqh); affine folded into ACT
                        rp1 = pt.tile([128, 512], BF16, tag="rp1")
                        act(rp1[:], d1s[:], AF.Abs_reciprocal_sqrt,
                            bias=c2[:], scale=-1.0)
                        x1 = px.tile([128, 512], BF16, tag="x1")
                        nc.vector.tensor_tensor(x1[:], Sc[:], rp1[:], ALU.mult)
                        x1_l.append(x1)

                    # phase 2: arctan L1 (trig set) + P1h + S2p/S2c
                    for jblk in range(N_JBLK):
                        Sc, sqh, x1 = Sc_l[jblk], sqh_l[jblk], x1_l[jblk]
                        at1 = pt.tile([128, 512], F32, tag="at1")
                        act(at1[:], x1[:], AF.Arctan)
                        P1h = px.tile([128, 512], BF16, tag="P1h")
                        nc.vector.scalar_tensor_tensor(
                            P1h[:], at1[:], PI / 4.0, Sc[:], ALU.add, ALU.mult
                        )
                        # S2c is half the reference S2 (constants refolded)
                        S2p = pt.tile([128, 512], BF16, tag="S2p")
                        nc.gpsimd.tensor_tensor(S2p[:], P1h[:], sqh[:], ALU.add)
                        S2c = px.tile([128, 512], BF16, tag="S2c")
                        nc.vector.tensor_scalar(S2c[:], S2p[:], 1.0,
                                                CLIP2 / 2.0, ALU.mult, ALU.min)
                        P1h_l.append(P1h); S2c_l.append(S2c)

                    # phase 3: t2/sq2 (sqrt set) + recip + x2
                    for jblk in range(N_JBLK):
                        S2c = S2c_l[jblk]
                        t2 = pt.tile([128, 512], F32, tag="t2")
                        nc.gpsimd.tensor_tensor(t2[:], S2c[:], S2c[:], ALU.mult)
                        sq2 = px.tile([128, 512], BF16, tag="sq2")
                        act(sq2[:], t2[:], AF.Sqrt,
                            bias=1.0, scale=-(4.0 * K_CONST * K_CONST))
                        d2s = px.tile([128, 512], BF16, tag="d2s")
                        nc.vector.scalar_tensor_tensor(
                            d2s[:], sq2[:], -2.0 / (4.0 * K_CONST * K_CONST),
                            t2[:], ALU.mult, ALU.add,
                        )
                        sq2_l.append(sq2); t2_l.append(d2s)

                    for jblk in range(N_JBLK):
                        S2c, sq2, d2s = S2c_l[jblk], sq2_l[jblk], t2_l[jblk]
                        # (1+sq2)^2 = 2 - 4K^2*(t2 - sq2/(2K^2))
                        rp2 = pt.tile([128, 512], BF16, tag="rp2")
                        act(rp2[:], d2s[:], AF.Abs_reciprocal_sqrt,
                            bias=c2[:], scale=-(4.0 * K_CONST * K_CONST))
                        x2 = px.tile([128, 512], BF16, tag="x2")
                        nc.vector.scalar_tensor_tensor(
                            x2[:], S2c[:], 2.0 * K_CONST, rp2[:], ALU.mult,
                            ALU.mult,
                        )
                        x2_l.append(x2)

                    # phase 4: arctan L2 (trig set) + G1h/m2h + reduction MMs
                    for jblk in range(N_JBLK):
                        P1h, S2c, sq2, x2 = (P1h_l[jblk], S2c_l[jblk],
                                             sq2_l[jblk], x2_l[jblk])
                        at2 = pt.tile([128, 512], F32, tag="at2")
                        act(at2[:], x2[:], AF.Arctan)
                        G1h = pt.tile([128, 512], BF16, tag="G1h")
                        nc.vector.scalar_tensor_tensor(
                            G1h[:], at2[:], PI / 4.0, P1h[:], ALU.add, ALU.mult
                        )
                        m2h = pt.tile([128, 512], BF16, tag="m2h")
                        nc.vector.scalar_tensor_tensor(
                            m2h[:], at2[:], PI / 4.0, S2c[:], ALU.add, ALU.mult
                        )
                        wslice = slice(NJ_DEV * jblk, NJ_DEV * (jblk + 1))
                        nc.tensor.matmul(
                            m1_ps[:], QD[:, wslice], G1h[:],
                            start=(jblk == 0), stop=(jblk == N_JBLK - 1),
                        )
                        nc.tensor.matmul(
                            m2_ps[:], WA[:, wslice], m2h[:],
                            start=(jblk == 0), stop=False,
                        )
                        nc.tensor.matmul(
                            m2_ps[:], WB[:, wslice], sq2[:],
                            start=False, stop=(jblk == N_JBLK - 1),
                        )

                    # finalize iblk: weighted a-reduction -> K_stage columns
                    pdf = pt.tile([1, 512], F32, tag="pdf")
                    prf = pt.tile([1, 512], F32, tag="prf")
                    nc.sync.dma_start(
                        pdf[:].rearrange("p (t s) -> p t s", s=128),
                        pdT[iblk * 4:(iblk + 1) * 4, :],
                    )
                    nc.sync.dma_start(
                        prf[:].rearrange("p (t s) -> p t s", s=128),
                        prT[iblk * 4:(iblk + 1) * 4, :],
                    )
                    pd_ps = pps.tile([NJ_DEV, 512], F32, tag="pdb_ps")
                    nc.tensor.matmul(pd_ps[:], ones_row[:, 0:NJ_DEV], pdf[:])
                    pdb = pt.tile([NJ_DEV, 512], F32, tag="pdb")
                    nc.scalar.copy(pdb[:], pd_ps[:])
                    pr_ps = pps.tile([NJ_DEV, 512], F32, tag="pdb_ps")
                    nc.tensor.matmul(pr_ps[:], ones_row[:, 0:NJ_DEV], prf[:])
                    prb = pt.tile([NJ_DEV, 512], F32, tag="prb")
                    nc.scalar.copy(prb[:], pr_ps[:])

                    w1 = pt.tile([NJ_DEV, 512], F32, tag="w1")
                    nc.vector.tensor_tensor(w1[:], m1_ps[:], pdb[:], ALU.mult)
                    w2 = pt.tile([NJ_DEV, 512], F32, tag="w2")
                    nc.vector.tensor_tensor(w2[:], m2_ps[:], prb[:], ALU.mult)
                    r1 = pt.tile([NJ_DEV, 8], F32, tag="r1")
                    nc.vector.tensor_reduce(
                        r1[:], w1[:].rearrange("p (j a) -> p j a", j=8),
                        mybir.AxisListType.X, ALU.add,
                    )
                    r2 = pt.tile([NJ_DEV, 8], F32, tag="r2")
                    nc.vector.tensor_reduce(
                        r2[:], w2[:].rearrange("p (j a) -> p j a", j=8),
                        mybir.AxisListType.X, ALU.add,
                    )
                    nc.vector.tensor_tensor(
                        K_stage[:, iblk * 8:(iblk + 1) * 8], r1[:], r2[:], ALU.add
                    )

            # serialize ACT transcendentals in emission order so the scheduler
            # cannot interleave table sets (each switch costs ~2.7us)
            from concourse.tile_rust import add_dep_helper
            for a, b in zip(act_chain[1:], act_chain[:-1]):
                add_dep_helper(a.ins, b.ins, reason="act table-set phase order")

            # ---------------- stage C: all-gather ----------------
            cc_in = dram.tile([NJ_DEV, N_S], F32)
            cc_out = dram.tile([NJ, N_S], F32, addr_space="Shared")
            nc.sync.dma_start(cc_in[:], K_stage[:])
            nc.gpsimd.collective_compute(
                "AllGather",
                ALU.bypass,
                ins=[cc_in[:].opt()],
                outs=[cc_out[:].opt()],
                replica_groups=[list(range(NCORES))],
            )

            # ---------------- stage D: solve + outputs ----------------
            with (
                tc.tile_pool(name="sol_sb", bufs=2) as ss,
                tc.tile_pool(name="sol_ps", bufs=1, space="PSUM") as sps,
            ):
                kssT = ss.tile([64, 64], F32, tag="kssT")
                nc.sync.dma_start(kssT[:], cc_out[0:64, :])
                kstT = ss.tile([128, 64], F32, tag="kstT")
                nc.sync.dma_start(kstT[:], cc_out[64:NJ, :])
                y_sb = ss.tile([64, 10], F32, tag="ysb")
                nc.sync.dma_start(y_sb[:], y_S[:])

                kss_ps = sps.tile([64, 64], F32, tag="kss_ps")
                nc.tensor.transpose(kss_ps[:], kssT[:], ident[0:64, 0:64])
                kss_sb = ss.tile([64, 64], F32, tag="kss_sb")
                nc.scalar.copy(kss_sb[:], kss_ps[:])
                nc.sync.dma_start(kss_out[:], kss_sb[:])

                # lambda = 1e-6 * tr(K_SS)/64, broadcast to [64,1]
                dd = ss.tile([64, 64], F32, tag="dd")
                nc.vector.tensor_tensor(dd[:], kss_sb[:], ident[0:64, 0:64],
                                        ALU.mult)
                dcol = ss.tile([64, 1], F32, tag="dcol")
                nc.vector.tensor_reduce(dcol[:], dd[:], mybir.AxisListType.X,
                                        ALU.add)
                from concourse import bass_isa
                tr_all = ss.tile([64, 1], F32, tag="tr_all")
                nc.gpsimd.partition_all_reduce(tr_all[:], dcol[:], 64,
                                               bass_isa.ReduceOp.add)
                lamcol = ss.tile([64, 1], F32, tag="lamcol")
                nc.vector.tensor_scalar(lamcol[:], tr_all[:], 1e-6 / 64.0,
                                        None, ALU.mult)

                aug = ss.tile([64, 74], F32, tag="aug")
                nc.vector.scalar_tensor_tensor(
                    aug[:, 0:64], ident[0:64, 0:64], lamcol[:], kss_sb[:],
                    ALU.mult, ALU.add,
                )
                nc.scalar.copy(aug[:, 64:74], y_sb[:])

                for k in range(64):
                    # broadcast row k of aug via a zero-stride one-hot lhsT:
                    # out[m,n] = sum_p ident[p,k] * aug[p,n] = aug[k,n]
                    row_ps = sps.tile([64, 74], F32, tag="row_ps")
                    nc.tensor.matmul(
                        row_ps[:],
                        ident[0:64, k:k + 1].broadcast_to([64, 64]),
                        aug[:, :],
                    )
                    rowb = ss.tile([64, 74], F32, tag="rowb")
                    nc.scalar.copy(rowb[:], row_ps[:])
                    rpiv = ss.tile([64, 1], F32, tag="rpiv")
                    nc.vector.reciprocal(rpiv[:], rowb[:, k:k + 1])
                    # nf = -aug[:,k]/piv with nf[k]=0, via negident col
                    # (negident[p,k] = -1 if p!=k else 0)
                    nf = ss.tile([64, 1], F32, tag="nf")
                    nc.vector.scalar_tensor_tensor(
                        nf[:], aug[:, k:k + 1], rpiv[:],
                        negident[0:64, k:k + 1], ALU.mult, ALU.mult,
                    )
                    aug_n = ss.tile([64, 74], F32, tag="aug")
                    nc.vector.scalar_tensor_tensor(
                        aug_n[:], rowb[:], nf[:], aug[:], ALU.mult, ALU.add
                    )
                    aug = aug_n

                dd2 = ss.tile([64, 64], F32, tag="dd")
                nc.vector.tensor_tensor(dd2[:], aug[:, 0:64], ident[0:64, 0:64],
                                        ALU.mult)
                dcol2 = ss.tile([64, 1], F32, tag="dcol")
                nc.vector.tensor_reduce(dcol2[:], dd2[:], mybir.AxisListType.X,
                                        ALU.add)
                rdg = ss.tile([64, 1], F32, tag="rdg")
                nc.vector.reciprocal(rdg[:], dcol2[:])
                Z = ss.tile([64, 10], F32, tag="Z")
                nc.vector.tensor_scalar(Z[:], aug[:, 64:74], rdg[:], None,
                                        ALU.mult)

                kst_ps = sps.tile([64, 128], F32, tag="kst_ps")
                nc.tensor.transpose(kst_ps[:], kstT[:], ident[:])
                kst_sb = ss.tile([64, 128], F32, tag="kst_sb")
                nc.scalar.copy(kst_sb[:], kst_ps[:])
                pred_ps = sps.tile([128, 10], F32, tag="pred_ps")
                nc.tensor.matmul(pred_ps[:], kst_sb[:], Z[:])
                pred_sb = ss.tile([128, 10], F32, tag="pred_sb")
                nc.scalar.copy(pred_sb[:], pred_ps[:])
                nc.sync.dma_start(pred_out[:], pred_sb[:])

    nc.compile()
    return nc


_PROGRAM = None


def _get_program():
    global _PROGRAM
    if _PROGRAM is None:
        _PROGRAM = build_program()
    return _PROGRAM


def make_in_maps(A_S, X_S, y_S, A_T, X_T):
    A_S = np.ascontiguousarray(A_S, dtype=np.float32)
    X_S = np.ascontiguousarray(X_S, dtype=np.float32)
    y_S = np.ascontiguousarray(y_S, dtype=np.float32)
    A_T = np.ascontiguousarray(A_T, dtype=np.float32)
    X_T = np.ascontiguousarray(X_T, dtype=np.float32)
    A_all = np.concatenate([A_S, A_T], axis=0)
    X_all = np.concatenate([X_S, X_T], axis=0)
    ident = np.eye(128, dtype=np.float32)
    epsident = (1e-4 * np.eye(128)).astype(np.float32)
    ones_row = np.ones((1, 128), dtype=np.float32)
    in_maps = []
    for d in range(NCORES):
        in_maps.append({
            "A_S": A_S, "X_S": X_S, "y_S": y_S,
            "A_J": A_all[d * NJ_DEV:(d + 1) * NJ_DEV],
            "X_J": X_all[d * NJ_DEV:(d + 1) * NJ_DEV],
            "ident": ident, "epsident": epsident, "ones_row": ones_row,
            "negident": (np.eye(128, dtype=np.float32) - 1.0),
        })
    return in_maps


def kernel(A_S, X_S, y_S, A_T, X_T):
    nc = _get_program()
    in_maps = make_in_maps(A_S, X_S, y_S, A_T, X_T)
    res = bass_utils.run_bass_kernel_spmd(
        nc, in_maps, core_ids=list(range(NCORES))
    )
    pred = np.asarray(res.results[0]["pred"], dtype=np.float32)
    kss = np.asarray(res.results[0]["K_SS"], dtype=np.float32)
    return pred, kss


# revision 25
# speedup vs baseline: 1.0537x; 1.0164x over previous
"""Trainium2 Bass kernel for nn_LiteNTK (graph NTK + ridge solve).

Contract: kernel(**inputs) takes FULL unsharded inputs and returns the FULL
output tuple (pred [128,10], K_SS [64,64]) matching reference.reference.

Math (validated against the jax reference in fp64/fp32/fp16 numpy models):

Per graph (A (64,64), X (64,64)), with A' = A + 1e-4 I:
  u = rowsum(A'), v = colsum(A'), Y = A' X, rn = row-norms^2 of Y
  d1 = sqrt((rn + 1e-4 u^2)/(u v))          # first-layer diag normalizer
  d2 = c*d1 with c^2 = Sn(0.9999) constant  # second-layer diag (S_diag == 1)
  left operand  YtilL^T = (A'/(u*d1) X)^T with aug row 1/d1
  right operand YtilR^T = (A'/(v*d1) X)^T with aug row 1e-4*u/(v*d1)
  p1 = A'^T (1/u), q2 = A'^T (1/v)          # reduction vectors

Pair (i left, j right): one K=65 matmul gives
  S1 = sigma0/(d1_i x d1_j)  directly.
Arc-cosine kernel recursion via half-angle arctan (ACT arctan domain is
[-pi/2, pi/2]; half-angle keeps |arg| <= 1):
  Sc = clip(S1), sq = sqrt(1-Sc^2), at = arctan(Sc/(1+sq)) = arcsin(Sc)/2
  P1h = (at+pi/4)*Sc                        # = pi*DS1*Sc/2
  S2p = 2*P1h + sq (= pi*Sn1), S2c = min(S2p, 0.9999*pi*c^2)
  sq2 = sqrt(1-K^2*S2c^2), at2 = arctan(K*S2c/(1+sq2)), K = 1/(pi*c^2)
  G1h = (at2+pi/4)*P1h, m2h = (at2+pi/4)*S2c
K[i,j] = pd_i . (4/pi^2 G1h) . qd_j + p_i . ((1+c^2)(2K/pi) m2h + sq2/pi) . q_j
         (all constants folded into per-graph weight vectors, /4096 split
          as /64 into each side)
Then KSS_reg = K_SS + 1e-6 tr/64 I, Gauss-Jordan solve (no pivoting; growth
factor 1.0 measured), pred = K_ST^T Z.

Sharding: combined j-axis (64 S + 128 T = 192 graphs) split 24 per core;
each core computes K[:, j_slice] (all 64 i), AllGather of the [24,64]
K-slices, then every core redundantly runs the tiny solve.
"""

import math
import sys

import numpy as np

for _p in ("/opt/trn_rl_repo",):
    if _p not in sys.path:
        sys.path.insert(0, _p)

import concourse.bacc as bacc
import concourse.bass as bass
import concourse.mybir as mybir
from concourse import bass_utils, tile

F32 = mybir.dt.float32
F16 = mybir.dt.float16
BF16 = mybir.dt.bfloat16
AF = mybir.ActivationFunctionType
ALU = mybir.AluOpType

NCORES = 8
N_S, N_T, NN, FEAT = 64, 128, 64, 64
NJ = N_S + N_T          # 192 combined j-graphs
NJ_DEV = NJ // NCORES   # 24 per core
N_SU = N_S // 2         # 32 S-side units (2 graphs each)
N_JU = NJ_DEV // 2      # 12 j-side units
N_IBLK = 8              # i blocks of 8 graphs (free dim 512)
N_JBLK = NJ_DEV // 2    # 12 j blocks of 2 graphs (partition dim 128)

PI = math.pi
A_CLIP = 0.9999
C2 = (A_CLIP * (PI - math.acos(A_CLIP)) + math.sqrt(1.0 - A_CLIP * A_CLIP)) / PI
K_CONST = 1.0 / (PI * C2)
CLIP2 = A_CLIP / K_CONST

# weight-vector constant folds (see module docstring)
QD_SCALE = 4.0 / (PI * PI) / 64.0
WA_SCALE = (1.0 + C2) * (4.0 * K_CONST / PI) / 64.0  # x2: m2h uses S2c' = S2c/2
WB_SCALE = (1.0 / PI) / 64.0
PD_SCALE = 1.0 / 64.0
P_SCALE = 1.0 / 64.0


def _emit_pass1(nc, sbuf, psum, cols, src_A, src_X, n_units, unit0,
                ident, epsident, cache):
    """Loads, transposes, Y-matmul, u/v/rn reduces. Caches abde/xst tiles."""
    for t in range(n_units):
        tc_col = unit0 + t
        abd = sbuf.tile([128, 128], F32, tag="abd")
        nc.vector.memset(abd[:], 0.0)
        nc.sync.dma_start(abd[0:64, 0:64], src_A[2 * t])
        nc.sync.dma_start(abd[64:128, 64:128], src_A[2 * t + 1])
        abde = sbuf.tile([128, 128], F32, tag="abde", bufs=46)
        nc.vector.tensor_tensor(abde[:], abd[:], epsident[:], ALU.add)

        xst = sbuf.tile([128, 64], F32, tag="xst", bufs=46)
        nc.sync.dma_start(xst[0:64, :], src_X[2 * t])
        nc.sync.dma_start(xst[64:128, :], src_X[2 * t + 1])

        nc.vector.tensor_reduce(
            cols["u"][:, tc_col:tc_col + 1], abde[:], mybir.AxisListType.X,
            ALU.add,
        )
        at_ps = psum.tile([128, 128], F32, tag="at_ps", bufs=2)
        nc.tensor.transpose(at_ps[:], abde[:], ident[:])
        abdT = sbuf.tile([128, 128], F32, tag="abdT")
        nc.scalar.copy(abdT[:], at_ps[:])
        nc.vector.tensor_reduce(
            cols["v"][:, tc_col:tc_col + 1], abdT[:], mybir.AxisListType.X,
            ALU.add,
        )
        y_ps = psum.tile([128, 64], F32, tag="y_ps", bufs=2)
        nc.tensor.matmul(y_ps[:], abdT[:], xst[:])
        ysq = sbuf.tile([128, 64], F32, tag="ysq")
        nc.scalar.activation(
            ysq[:], y_ps[:], AF.Square,
            accum_out=cols["rn"][:, tc_col:tc_col + 1],
        )
        cache.append((abde, xst))


def _emit_col_math(nc, sbuf, cols, n_all):
    """Batched [128, n_all] column math: d1 and all derived scale vectors."""
    A = slice(0, n_all)
    uu = sbuf.tile([128, n_all], F32, tag="b_uu")
    nc.vector.tensor_scalar(uu[:], cols["u"][:, A], 1e-4, None, ALU.mult)
    rn2 = sbuf.tile([128, n_all], F32, tag="b_rn2")
    nc.vector.tensor_tensor(rn2[:], uu[:], cols["u"][:, A], ALU.mult)
    nc.vector.tensor_tensor(rn2[:], rn2[:], cols["rn"][:, A], ALU.add)
    uv = sbuf.tile([128, n_all], F32, tag="b_uv")
    nc.vector.tensor_tensor(uv[:], cols["u"][:, A], cols["v"][:, A], ALU.mult)
    ruv = sbuf.tile([128, n_all], F32, tag="b_ruv")
    nc.vector.reciprocal(ruv[:], uv[:])
    rat = sbuf.tile([128, n_all], F32, tag="b_rat")
    nc.vector.tensor_tensor(rat[:], rn2[:], ruv[:], ALU.mult)
    nc.scalar.activation(cols["d1"][:, A], rat[:], AF.Sqrt)
    ud1 = sbuf.tile([128, n_all], F32, tag="b_ud1")
    nc.vector.tensor_tensor(ud1[:], cols["u"][:, A], cols["d1"][:, A], ALU.mult)
    nc.vector.reciprocal(cols["sL"][:, A], ud1[:])
    vd1 = sbuf.tile([128, n_all], F32, tag="b_vd1")
    nc.vector.tensor_tensor(vd1[:], cols["v"][:, A], cols["d1"][:, A], ALU.mult)
    nc.vector.reciprocal(cols["sR"][:, A], vd1[:])
    nc.vector.reciprocal(cols["ru"][:, A], cols["u"][:, A])
    nc.vector.reciprocal(cols["rv"][:, A], cols["v"][:, A])
    nc.vector.reciprocal(cols["rd1"][:, A], cols["d1"][:, A])


def _emit_pass3(nc, sbuf, psum, cols, n_units, unit0, ident, cache,
                need_left, dst_L, dst_R):
    """Per-unit: p1/q2 matmuls into col tiles + scaled operand build."""
    for t in range(n_units):
        tc_col = unit0 + t
        abde, xst = cache[t]
        if need_left:
            p1_ps = psum.tile([128, 1], F32, tag="p1_ps")
            nc.tensor.matmul(p1_ps[:], abde[:],
                             cols["ru"][:, tc_col:tc_col + 1])
            nc.scalar.copy(cols["p1"][:, tc_col:tc_col + 1], p1_ps[:])
            scale = cols["sL"][:, tc_col:tc_col + 1]
            dst = dst_L
        else:
            q2_ps = psum.tile([128, 1], F32, tag="p1_ps")
            nc.tensor.matmul(q2_ps[:], abde[:],
                             cols["rv"][:, tc_col:tc_col + 1])
            nc.scalar.copy(cols["q2c"][:, t:t + 1], q2_ps[:])
            scale = cols["sR"][:, tc_col:tc_col + 1]
            dst = dst_R
        atl = sbuf.tile([128, 128], F32, tag="atl")
        nc.vector.tensor_scalar(atl[:], abde[:], scale, None, ALU.mult)
        tl_ps = psum.tile([128, 128], F32, tag="tl_ps", bufs=2)
        nc.tensor.transpose(tl_ps[:], atl[:], ident[:])
        tl_sb = sbuf.tile([128, 128], F32, tag="tl_sb")
        nc.scalar.copy(tl_sb[:], tl_ps[:])
        ytl_ps = psum.tile([64, 128], F32, tag="ytl_ps")
        nc.tensor.matmul(ytl_ps[:], xst[:], tl_sb[:])
        nc.scalar.copy(dst[0:64, 128 * t:128 * (t + 1)], ytl_ps[:])


def build_program():
    nc = bacc.Bacc("TRN2", target_bir_lowering=False, debug=False,
                   enable_asserts=False, num_devices=NCORES)

    A_S = nc.dram_tensor("A_S", [N_S, NN, NN], F32, kind="ExternalInput").ap()
    X_S = nc.dram_tensor("X_S", [N_S, NN, FEAT], F32, kind="ExternalInput").ap()
    y_S = nc.dram_tensor("y_S", [N_S, 10], F32, kind="ExternalInput").ap()
    A_J = nc.dram_tensor("A_J", [NJ_DEV, NN, NN], F32, kind="ExternalInput").ap()
    X_J = nc.dram_tensor("X_J", [NJ_DEV, NN, FEAT], F32, kind="ExternalInput").ap()
    ident_in = nc.dram_tensor("ident", [128, 128], F32, kind="ExternalInput").ap()
    epsident_in = nc.dram_tensor("epsident", [128, 128], F32, kind="ExternalInput").ap()
    ones_in = nc.dram_tensor("ones_row", [1, 128], F32, kind="ExternalInput").ap()
    negident_in = nc.dram_tensor("negident", [128, 128], F32,
                                 kind="ExternalInput").ap()

    pred_out = nc.dram_tensor("pred", [N_T, 10], F32, kind="ExternalOutput").ap()
    kss_out = nc.dram_tensor("K_SS", [N_S, N_S], F32, kind="ExternalOutput").ap()

    with tile.TileContext(nc) as tc:
        with (
            tc.tile_pool(name="persist", bufs=1) as pp,
            tc.tile_pool(name="dram", bufs=1, space="DRAM") as dram,
        ):
            ident = pp.tile([128, 128], F32)
            epsident = pp.tile([128, 128], F32)
            ones_row = pp.tile([1, 128], F32)
            negident = pp.tile([128, 128], F32)
            nc.sync.dma_start(ident[:], ident_in[:])
            nc.sync.dma_start(epsident[:], epsident_in[:])
            nc.sync.dma_start(ones_row[:], ones_in[:])
            nc.sync.dma_start(negident[:], negident_in[:])

            c025 = pp.tile([128, 1], F32)
            nc.vector.memset(c025[:], 0.25)
            c2 = pp.tile([128, 1], F32)
            nc.vector.memset(c2[:], 2.0)
            RHS_L = pp.tile([65, N_S * NN], F32)      # i-side stacked YtilL^T+aug
            LHS_R = pp.tile([65, NJ_DEV * NN], F32)   # j-side stacked YtilR^T+aug
            # per-jblk expanded weight tiles: slice m is [128, 24] with only
            # columns 2m (partitions 0:64) and 2m+1 (partitions 64:128) nonzero
            QD = pp.tile([128, NJ_DEV * N_JBLK], BF16)
            WA = pp.tile([128, NJ_DEV * N_JBLK], BF16)
            WB = pp.tile([128, NJ_DEV * N_JBLK], BF16)
            K_stage = pp.tile([NJ_DEV, N_S], F32)

            cols = {}
            for name in ("u", "v", "rn", "d1", "rd1", "pd", "p", "sL", "sR",
                         "ru", "rv", "p1"):
                cols[name] = pp.tile([128, N_SU + N_JU], F32, name=f"col_{name}")
            for name in ("qdw", "waw", "wbw", "augj", "q2c"):
                cols[name] = pp.tile([128, N_JU], F32, name=f"col_{name}")

            # ---------------- stage A: per-graph precompute ----------------
            with (
                tc.tile_pool(name="pre_sb", bufs=6) as sbuf,
                tc.tile_pool(name="pre_ps", bufs=1, space="PSUM") as psum,
            ):
                cache_S, cache_J = [], []
                _emit_pass1(nc, sbuf, psum, cols, A_S, X_S, N_SU, 0,
                            ident, epsident, cache_S)
                _emit_pass1(nc, sbuf, psum, cols, A_J, X_J, N_JU, N_SU,
                            ident, epsident, cache_J)
                _emit_col_math(nc, sbuf, cols, N_SU + N_JU)
                _emit_pass3(nc, sbuf, psum, cols, N_SU, 0, ident, cache_S,
                            True, RHS_L, None)
                _emit_pass3(nc, sbuf, psum, cols, N_JU, N_SU, ident, cache_J,
                            False, None, LHS_R)
                # batched weight-vector math
                SL, JL = slice(0, N_SU), slice(N_SU, N_SU + N_JU)
                nc.vector.scalar_tensor_tensor(
                    cols["pd"][:, SL], cols["p1"][:, SL], PD_SCALE,
                    cols["d1"][:, SL], ALU.mult, ALU.mult,
                )
                nc.vector.tensor_scalar(
                    cols["p"][:, SL], cols["p1"][:, SL], P_SCALE, None, ALU.mult
                )
                nc.vector.scalar_tensor_tensor(
                    cols["qdw"][:, :], cols["q2c"][:, :], QD_SCALE,
                    cols["d1"][:, JL], ALU.mult, ALU.mult,
                )
                nc.vector.tensor_scalar(
                    cols["waw"][:, :], cols["q2c"][:, :], WA_SCALE, None,
                    ALU.mult,
                )
                nc.vector.tensor_scalar(
                    cols["wbw"][:, :], cols["q2c"][:, :], WB_SCALE, None,
                    ALU.mult,
                )
                nc.vector.scalar_tensor_tensor(
                    cols["augj"][:, :], cols["u"][:, JL], 1e-4,
                    cols["sR"][:, JL], ALU.mult, ALU.mult,
                )

            # aug rows: RHS_L[64, 128*t + suba] = rd1cols[suba, t].
            # Engines can't move data across partitions, so transpose the
            # column tiles on the PE first, then DMA the row-major result.
            with tc.tile_pool(name="augt_ps", bufs=1, space="PSUM") as aps:
                rd1T_ps = aps.tile([N_SU, 128], F32, tag="rd1T_ps")
                nc.tensor.transpose(rd1T_ps[:], cols["rd1"][:, 0:N_SU],
                                    ident[:])
                rd1T = pp.tile([N_SU, 128], F32)
                nc.scalar.copy(rd1T[:], rd1T_ps[:])
                augjT_ps = aps.tile([N_JU, 128], F32, tag="augjT_ps")
                nc.tensor.transpose(augjT_ps[:], cols["augj"][:, 0:N_JU],
                                    ident[:])
                augjT = pp.tile([N_JU, 128], F32)
                nc.scalar.copy(augjT[:], augjT_ps[:])
                pdT_ps = aps.tile([N_SU, 128], F32, tag="rd1T_ps")
                nc.tensor.transpose(pdT_ps[:], cols["pd"][:, 0:N_SU], ident[:])
                pdT = pp.tile([N_SU, 128], F32)
                nc.scalar.copy(pdT[:], pdT_ps[:])
                prT_ps = aps.tile([N_SU, 128], F32, tag="rd1T_ps")
                nc.tensor.transpose(prT_ps[:], cols["p"][:, 0:N_SU], ident[:])
                prT = pp.tile([N_SU, 128], F32)
                nc.scalar.copy(prT[:], prT_ps[:])
            nc.sync.dma_start(
                RHS_L[64:65, :].rearrange("p (t s) -> p t s", s=128), rd1T[:]
            )
            nc.sync.dma_start(
                LHS_R[64:65, :].rearrange("p (t s) -> p t s", s=128), augjT[:]
            )
            # weight block-columns (expanded per jblk for psum-accumulating
            # reduction matmuls): even j -> partitions 0:64 col 24m+2m,
            # odd j -> partitions 64:128 col 24m+2m+1
            for w_all, w_col in ((QD, "qdw"), (WA, "waw"), (WB, "wbw")):
                nc.vector.memset(w_all[:], 0.0)
                for m in range(N_JBLK):
                    base = NJ_DEV * m + 2 * m
                    nc.scalar.copy(w_all[0:64, base:base + 1],
                                   cols[w_col][0:64, m:m + 1])
                    nc.scalar.copy(w_all[64:128, base + 1:base + 2],
                                   cols[w_col][64:128, m:m + 1])

            # ---------------- stage B: pair loop ----------------
            # Engine placement tuned from HW microbenches (ns per [128,512]):
            #   DVE: TT 475, TS 269, STT 602, clip-from-psum 599, recip 3212
            #   GPS: TT ~1279, TS(mult/add dual) 680; min on GPS = 7.4us (!)
            #   ACT: 720/op + ~2.7us per table-set switch (phase-batched)
            # bf16 for phase-crossing storage; fp32 for t1/t2 (cancellation
            # near |S|~1) and arctan outputs.
            act_chain = []

            def act(*args, **kw):
                inst = nc.scalar.activation(*args, **kw)
                act_chain.append(inst)
                return inst

            with (
                tc.tile_pool(name="pb_x", bufs=13) as px,     # phase-crossing
                tc.tile_pool(name="pb_t", bufs=2) as pt,      # in-phase temps
                tc.tile_pool(name="pb_ps", bufs=2, space="PSUM") as pps,
                tc.tile_pool(name="pb_mps", bufs=2, space="PSUM") as mps,
            ):
                for iblk in range(N_IBLK):
                    i0 = iblk * 512
                    m1_ps = mps.tile([NJ_DEV, 512], F32, tag="m1")
                    m2_ps = mps.tile([NJ_DEV, 512], F32, tag="m2")
                    Sc_l, sqh_l, x1_l, t1_l = [], [], [], []
                    P1h_l, S2c_l, sq2_l, x2_l, t2_l = [], [], [], [], []

                    # phase 1: matmul + clip + t1/sqh (sqrt set) + recip + x1
                    for jblk in range(N_JBLK):
                        s1_ps = pps.tile([128, 512], F32, tag="s1")
                        nc.tensor.matmul(
                            s1_ps[:],
                            LHS_R[:, jblk * 128:(jblk + 1) * 128],
                            RHS_L[:, i0:i0 + 512],
                        )
                        Sc = px.tile([128, 512], BF16, tag="Sc")
                        nc.vector.tensor_scalar(
                            Sc[:], s1_ps[:], -A_CLIP, A_CLIP, ALU.max, ALU.min
                        )
                        t1 = pt.tile([128, 512], F32, tag="t1")
                        nc.gpsimd.tensor_tensor(t1[:], Sc[:], Sc[:], ALU.mult)
                        sqh = px.tile([128, 512], BF16, tag="sqh")
                        act(sqh[:], t1[:], AF.Sqrt, bias=c025[:], scale=-0.25)
                        d1s = px.tile([128, 512], BF16, tag="d1s")
                        nc.vector.scalar_tensor_tensor(
                            d1s[:], sqh[:], -4.0, t1[:], ALU.mult, ALU.add
                        )
                        Sc_l.append(Sc); sqh_l.append(sqh); t1_l.append(d1s)

                    for jblk in range(N_JBLK):
                        Sc, sqh, d1s = Sc_l[jblk], sqh_l[jblk], t1_l[jblk]
                        # (1+sq)^2 = 2 - (t1 - 4*sqh); affine folded into ACT
                        rp1 = pt.tile([128, 512], BF16, tag="rp1")
                        act(rp1[:], d1s[:], AF.Abs_reciprocal_sqrt,
                            bias=c2[:], scale=-1.0)
                        x1 = px.tile([128, 512], BF16, tag="x1")
                        nc.gpsimd.tensor_tensor(x1[:], Sc[:], rp1[:], ALU.mult)
                        x1_l.append(x1)

                    # phase 2: arctan L1 (trig set) + P1h + S2p/S2c
                    for jblk in range(N_JBLK):
                        Sc, sqh, x1 = Sc_l[jblk], sqh_l[jblk], x1_l[jblk]
                        at1 = pt.tile([128, 512], F32, tag="at1")
                        act(at1[:], x1[:], AF.Arctan)
                        P1h = px.tile([128, 512], BF16, tag="P1h")
                        nc.vector.scalar_tensor_tensor(
                            P1h[:], at1[:], PI / 4.0, Sc[:], ALU.add, ALU.mult
                        )
                        # S2c is half the reference S2 (constants refolded)
                        S2p = pt.tile([128, 512], BF16, tag="S2p")
                        nc.gpsimd.tensor_tensor(S2p[:], P1h[:], sqh[:], ALU.add)
                        S2c = px.tile([128, 512], BF16, tag="S2c")
                        nc.vector.tensor_scalar(S2c[:], S2p[:], 1.0,
                                                CLIP2 / 2.0, ALU.mult, ALU.min)
                        P1h_l.append(P1h); S2c_l.append(S2c)

                    # phase 3: t2/sq2 (sqrt set) + recip + x2
                    for jblk in range(N_JBLK):
                        S2c = S2c_l[jblk]
                        t2 = pt.tile([128, 512], F32, tag="t2")
                        nc.gpsimd.tensor_tensor(t2[:], S2c[:], S2c[:], ALU.mult)
                        sq2 = px.tile([128, 512], BF16, tag="sq2")
                        act(sq2[:], t2[:], AF.Sqrt,
                            bias=1.0, scale=-(4.0 * K_CONST * K_CONST))
                        d2s = px.tile([128, 512], BF16, tag="d2s")
                        nc.vector.scalar_tensor_tensor(
                            d2s[:], sq2[:], -2.0 / (4.0 * K_CONST * K_CONST),
                            t2[:], ALU.mult, ALU.add,
                        )
                        sq2_l.append(sq2); t2_l.append(d2s)

                    for jblk in range(N_JBLK):
                        S2c, sq2, d2s = S2c_l[jblk], sq2_l[jblk], t2_l[jblk]
                        # (1+sq2)^2 = 2 - 4K^2*(t2 - sq2/(2K^2))
                        rp2 = pt.tile([128, 512], BF16, tag="rp2")
                        act(rp2[:], d2s[:], AF.Abs_reciprocal_sqrt,
                            bias=c2[:], scale=-(4.0 * K_CONST * K_CONST))
                        x2 = px.tile([128, 512], BF16, tag="x2")
                        nc.vector.scalar_tensor_tensor(
                            x2[:], S2c[:], 2.0 * K_CONST, rp2[:], ALU.mult,
                            ALU.mult,
                        )
                        x2_l.append(x2)

                    # phase 4: arctan L2 (trig set) + G1h/m2h + reduction MMs
                    for jblk in range(N_JBLK):
                        P1h, S2c, sq2, x2 = (P1h_l[jblk], S2c_l[jblk],
                                             sq2_l[jblk], x2_l[jblk])
                        at2 = pt.tile([128, 512], F32, tag="at2")
                        act(at2[:], x2[:], AF.Arctan)
                        G1h = pt.tile([128, 512], BF16, tag="G1h")
                        nc.vector.scalar_tensor_tensor(
                            G1h[:], at2[:], PI / 4.0, P1h[:], ALU.add, ALU.mult
                        )
                        m2h = pt.tile([128, 512], BF16, tag="m2h")
                        nc.vector.scalar_tensor_tensor(
                            m2h[:], at2[:], PI / 4.0, S2c[:], ALU.add, ALU.mult
                        )
                        wslice = slice(NJ_DEV * jblk, NJ_DEV * (jblk + 1))
                        nc.tensor.matmul(
                            m1_ps[:], QD[:, wslice], G1h[:],
                            start=(jblk == 0), stop=(jblk == N_JBLK - 1),
                        )
                        nc.tensor.matmul(
                            m2_ps[:], WA[:, wslice], m2h[:],
                            start=(jblk == 0), stop=False,
                        )
                        nc.tensor.matmul(
                            m2_ps[:], WB[:, wslice], sq2[:],
                            start=False, stop=(jblk == N_JBLK - 1),
                        )

                    # finalize iblk: weighted a-reduction -> K_stage columns
                    pdf = pt.tile([1, 512], F32, tag="pdf")
                    prf = pt.tile([1, 512], F32, tag="prf")
                    nc.sync.dma_start(
                        pdf[:].rearrange("p (t s) -> p t s", s=128),
                        pdT[iblk * 4:(iblk + 1) * 4, :],
                    )
                    nc.sync.dma_start(
                        prf[:].rearrange("p (t s) -> p t s", s=128),
                        prT[iblk * 4:(iblk + 1) * 4, :],
                    )
                    pd_ps = pps.tile([NJ_DEV, 512], F32, tag="pdb_ps")
                    nc.tensor.matmul(pd_ps[:], ones_row[:, 0:NJ_DEV], pdf[:])
                    pdb = pt.tile([NJ_DEV, 512], F32, tag="pdb")
                    nc.scalar.copy(pdb[:], pd_ps[:])
                    pr_ps = pps.tile([NJ_DEV, 512], F32, tag="pdb_ps")
                    nc.tensor.matmul(pr_ps[:], ones_row[:, 0:NJ_DEV], prf[:])
                    prb = pt.tile([NJ_DEV, 512], F32, tag="prb")
                    nc.scalar.copy(prb[:], pr_ps[:])

                    w1 = pt.tile([NJ_DEV, 512], F32, tag="w1")
                    nc.vector.tensor_tensor(w1[:], m1_ps[:], pdb[:], ALU.mult)
                    w2 = pt.tile([NJ_DEV, 512], F32, tag="w2")
                    nc.vector.tensor_tensor(w2[:], m2_ps[:], prb[:], ALU.mult)
                    r1 = pt.tile([NJ_DEV, 8], F32, tag="r1")
                    nc.vector.tensor_reduce(
                        r1[:], w1[:].rearrange("p (j a) -> p j a", j=8),
                        mybir.AxisListType.X, ALU.add,
                    )
                    r2 = pt.tile([NJ_DEV, 8], F32, tag="r2")
                    nc.vector.tensor_reduce(
                        r2[:], w2[:].rearrange("p (j a) -> p j a", j=8),
                        mybir.AxisListType.X, ALU.add,
                    )
                    nc.vector.tensor_tensor(
                        K_stage[:, iblk * 8:(iblk + 1) * 8], r1[:], r2[:], ALU.add
                    )

            # serialize ACT transcendentals in emission order so the scheduler
            # cannot interleave table sets (each switch costs ~2.7us)
            from concourse.tile_rust import add_dep_helper
            for a, b in zip(act_chain[1:], act_chain[:-1]):
                add_dep_helper(a.ins, b.ins, reason="act table-set phase order")

            # ---------------- stage C: all-gather ----------------
            cc_in = dram.tile([NJ_DEV, N_S], F32)
            cc_out = dram.tile([NJ, N_S], F32, addr_space="Shared")
            nc.sync.dma_start(cc_in[:], K_stage[:])
            nc.gpsimd.collective_compute(
                "AllGather",
                ALU.bypass,
                ins=[cc_in[:].opt()],
                outs=[cc_out[:].opt()],
                replica_groups=[list(range(NCORES))],
            )

            # ---------------- stage D: solve + outputs ----------------
            with (
                tc.tile_pool(name="sol_sb", bufs=2) as ss,
                tc.tile_pool(name="sol_ps", bufs=1, space="PSUM") as sps,
            ):
                kssT = ss.tile([64, 64], F32, tag="kssT")
                nc.sync.dma_start(kssT[:], cc_out[0:64, :])
                kstT = ss.tile([128, 64], F32, tag="kstT")
                nc.sync.dma_start(kstT[:], cc_out[64:NJ, :])
                y_sb = ss.tile([64, 10], F32, tag="ysb")
                nc.sync.dma_start(y_sb[:], y_S[:])

                kss_ps = sps.tile([64, 64], F32, tag="kss_ps")
                nc.tensor.transpose(kss_ps[:], kssT[:], ident[0:64, 0:64])
                kss_sb = ss.tile([64, 64], F32, tag="kss_sb")
                nc.scalar.copy(kss_sb[:], kss_ps[:])
                nc.sync.dma_start(kss_out[:], kss_sb[:])

                # lambda = 1e-6 * tr(K_SS)/64, broadcast to [64,1]
                dd = ss.tile([64, 64], F32, tag="dd")
                nc.vector.tensor_tensor(dd[:], kss_sb[:], ident[0:64, 0:64],
                                        ALU.mult)
                dcol = ss.tile([64, 1], F32, tag="dcol")
                nc.vector.tensor_reduce(dcol[:], dd[:], mybir.AxisListType.X,
                                        ALU.add)
                from concourse import bass_isa
                tr_all = ss.tile([64, 1], F32, tag="tr_all")
                nc.gpsimd.partition_all_reduce(tr_all[:], dcol[:], 64,
                                               bass_isa.ReduceOp.add)
                lamcol = ss.tile([64, 1], F32, tag="lamcol")
                nc.vector.tensor_scalar(lamcol[:], tr_all[:], 1e-6 / 64.0,
                                        None, ALU.mult)

                aug = ss.tile([64, 74], F32, tag="aug")
                nc.vector.scalar_tensor_tensor(
                    aug[:, 0:64], ident[0:64, 0:64], lamcol[:], kss_sb[:],
                    ALU.mult, ALU.add,
                )
                nc.scalar.copy(aug[:, 64:74], y_sb[:])

                for k in range(64):
                    # broadcast row k of aug via a zero-stride one-hot lhsT:
                    # out[m,n] = sum_p ident[p,k] * aug[p,n] = aug[k,n]
                    row_ps = sps.tile([64, 74], F32, tag="row_ps", bufs=2)
                    nc.tensor.matmul(
                        row_ps[:],
                        ident[0:64, k:k + 1].broadcast_to([64, 64]),
                        aug[:, :],
                    )
                    rpiv = ss.tile([64, 1], F32, tag="rpiv")
                    nc.vector.reciprocal(rpiv[:], row_ps[:, k:k + 1])
                    # nf = -aug[:,k]/piv with nf[k]=0, via negident col
                    # (negident[p,k] = -1 if p!=k else 0)
                    nf = ss.tile([64, 1], F32, tag="nf")
                    nc.vector.scalar_tensor_tensor(
                        nf[:], aug[:, k:k + 1], rpiv[:],
                        negident[0:64, k:k + 1], ALU.mult, ALU.mult,
                    )
                    aug_n = ss.tile([64, 74], F32, tag="aug")
                    nc.vector.scalar_tensor_tensor(
                        aug_n[:], row_ps[:], nf[:], aug[:], ALU.mult, ALU.add
                    )
                    aug = aug_n

                dd2 = ss.tile([64, 64], F32, tag="dd")
                nc.vector.tensor_tensor(dd2[:], aug[:, 0:64], ident[0:64, 0:64],
                                        ALU.mult)
                dcol2 = ss.tile([64, 1], F32, tag="dcol")
                nc.vector.tensor_reduce(dcol2[:], dd2[:], mybir.AxisListType.X,
                                        ALU.add)
                rdg = ss.tile([64, 1], F32, tag="rdg")
                nc.vector.reciprocal(rdg[:], dcol2[:])
                Z = ss.tile([64, 10], F32, tag="Z")
                nc.vector.tensor_scalar(Z[:], aug[:, 64:74], rdg[:], None,
                                        ALU.mult)

                kst_ps = sps.tile([64, 128], F32, tag="kst_ps")
                nc.tensor.transpose(kst_ps[:], kstT[:], ident[:])
                kst_sb = ss.tile([64, 128], F32, tag="kst_sb")
                nc.scalar.copy(kst_sb[:], kst_ps[:])
                pred_ps = sps.tile([128, 10], F32, tag="pred_ps")
                nc.tensor.matmul(pred_ps[:], kst_sb[:], Z[:])
                pred_sb = ss.tile([128, 10], F32, tag="pred_sb")
                nc.scalar.copy(pred_sb[:], pred_ps[:])
                nc.sync.dma_start(pred_out[:], pred_sb[:])

    nc.compile()
    return nc


_PROGRAM = None


def _get_program():
    global _PROGRAM
    if _PROGRAM is None:
        _PROGRAM = build_program()
    return _PROGRAM


def make_in_maps(A_S, X_S, y_S, A_T, X_T):
    A_S = np.ascontiguousarray(A_S, dtype=np.float32)
    X_S = np.ascontiguousarray(X_S, dtype=np.float32)
    y_S = np.ascontiguousarray(y_S, dtype=np.float32)
    A_T = np.ascontiguousarray(A_T, dtype=np.float32)
    X_T = np.ascontiguousarray(X_T, dtype=np.float32)
    A_all = np.concatenate([A_S, A_T], axis=0)
    X_all = np.concatenate([X_S, X_T], axis=0)
    ident = np.eye(128, dtype=np.float32)
    epsident = (1e-4 * np.eye(128)).astype(np.float32)
    ones_row = np.ones((1, 128), dtype=np.float32)
    in_maps = []
    for d in range(NCORES):
        in_maps.append({
            "A_S": A_S, "X_S": X_S, "y_S": y_S,
            "A_J": A_all[d * NJ_DEV:(d + 1) * NJ_DEV],
            "X_J": X_all[d * NJ_DEV:(d + 1) * NJ_DEV],
            "ident": ident, "epsident": epsident, "ones_row": ones_row,
            "negident": (np.eye(128, dtype=np.float32) - 1.0),
        })
    return in_maps


def kernel(A_S, X_S, y_S, A_T, X_T):
    nc = _get_program()
    in_maps = make_in_maps(A_S, X_S, y_S, A_T, X_T)
    res = bass_utils.run_bass_kernel_spmd(
        nc, in_maps, core_ids=list(range(NCORES))
    )
    pred = np.asarray(res.results[0]["pred"], dtype=np.float32)
    kss = np.asarray(res.results[0]["K_SS"], dtype=np.float32)
    return pred, kss


# revision 26
# speedup vs baseline: 1.0785x; 1.0236x over previous
"""Trainium2 Bass kernel for nn_LiteNTK (graph NTK + ridge solve).

Contract: kernel(**inputs) takes FULL unsharded inputs and returns the FULL
output tuple (pred [128,10], K_SS [64,64]) matching reference.reference.

Math (validated against the jax reference in fp64/fp32/fp16 numpy models):

Per graph (A (64,64), X (64,64)), with A' = A + 1e-4 I:
  u = rowsum(A'), v = colsum(A'), Y = A' X, rn = row-norms^2 of Y
  d1 = sqrt((rn + 1e-4 u^2)/(u v))          # first-layer diag normalizer
  d2 = c*d1 with c^2 = Sn(0.9999) constant  # second-layer diag (S_diag == 1)
  left operand  YtilL^T = (A'/(u*d1) X)^T with aug row 1/d1
  right operand YtilR^T = (A'/(v*d1) X)^T with aug row 1e-4*u/(v*d1)
  p1 = A'^T (1/u), q2 = A'^T (1/v)          # reduction vectors

Pair (i left, j right): one K=65 matmul gives
  S1 = sigma0/(d1_i x d1_j)  directly.
Arc-cosine kernel recursion via half-angle arctan (ACT arctan domain is
[-pi/2, pi/2]; half-angle keeps |arg| <= 1):
  Sc = clip(S1), sq = sqrt(1-Sc^2), at = arctan(Sc/(1+sq)) = arcsin(Sc)/2
  P1h = (at+pi/4)*Sc                        # = pi*DS1*Sc/2
  S2p = 2*P1h + sq (= pi*Sn1), S2c = min(S2p, 0.9999*pi*c^2)
  sq2 = sqrt(1-K^2*S2c^2), at2 = arctan(K*S2c/(1+sq2)), K = 1/(pi*c^2)
  G1h = (at2+pi/4)*P1h, m2h = (at2+pi/4)*S2c
K[i,j] = pd_i . (4/pi^2 G1h) . qd_j + p_i . ((1+c^2)(2K/pi) m2h + sq2/pi) . q_j
         (all constants folded into per-graph weight vectors, /4096 split
          as /64 into each side)
Then KSS_reg = K_SS + 1e-6 tr/64 I, Gauss-Jordan solve (no pivoting; growth
factor 1.0 measured), pred = K_ST^T Z.

Sharding: combined j-axis (64 S + 128 T = 192 graphs) split 24 per core;
each core computes K[:, j_slice] (all 64 i), AllGather of the [24,64]
K-slices, then every core redundantly runs the tiny solve.
"""

import math
import sys

import numpy as np

for _p in ("/opt/trn_rl_repo",):
    if _p not in sys.path:
        sys.path.insert(0, _p)

import concourse.bacc as bacc
import concourse.bass as bass
import concourse.mybir as mybir
from concourse import bass_utils, tile

F32 = mybir.dt.float32
F16 = mybir.dt.float16
BF16 = mybir.dt.bfloat16
AF = mybir.ActivationFunctionType
ALU = mybir.AluOpType

NCORES = 8
N_S, N_T, NN, FEAT = 64, 128, 64, 64
NJ = N_S + N_T          # 192 combined j-graphs
NJ_DEV = NJ // NCORES   # 24 per core
N_SU = N_S // 2         # 32 S-side units (2 graphs each)
N_JU = NJ_DEV // 2      # 12 j-side units
N_IBLK = 8              # i blocks of 8 graphs (free dim 512)
N_JBLK = NJ_DEV // 2    # 12 j blocks of 2 graphs (partition dim 128)

PI = math.pi
A_CLIP = 0.9999
C2 = (A_CLIP * (PI - math.acos(A_CLIP)) + math.sqrt(1.0 - A_CLIP * A_CLIP)) / PI
K_CONST = 1.0 / (PI * C2)
CLIP2 = A_CLIP / K_CONST

# weight-vector constant folds (see module docstring)
QD_SCALE = 4.0 / (PI * PI) / 64.0
WA_SCALE = (1.0 + C2) * (4.0 * K_CONST / PI) / 64.0  # x2: m2h uses S2c' = S2c/2
WB_SCALE = (1.0 / PI) / 64.0
PD_SCALE = 1.0 / 64.0
P_SCALE = 1.0 / 64.0


def _emit_pass1(nc, sbuf, psum, cols, src_A, src_X, n_units, unit0,
                ident, epsident, cache):
    """Loads, transposes, Y-matmul, u/v/rn reduces. Caches abde/xst tiles."""
    for t in range(n_units):
        tc_col = unit0 + t
        abd = sbuf.tile([128, 128], F32, tag="abd")
        nc.vector.memset(abd[:], 0.0)
        nc.sync.dma_start(abd[0:64, 0:64], src_A[2 * t])
        nc.sync.dma_start(abd[64:128, 64:128], src_A[2 * t + 1])
        abde = sbuf.tile([128, 128], F32, tag="abde", bufs=46)
        nc.vector.tensor_tensor(abde[:], abd[:], epsident[:], ALU.add)

        xst = sbuf.tile([128, 64], F32, tag="xst")
        nc.sync.dma_start(xst[0:64, :], src_X[2 * t])
        nc.sync.dma_start(xst[64:128, :], src_X[2 * t + 1])

        nc.vector.tensor_reduce(
            cols["u"][:, tc_col:tc_col + 1], abde[:], mybir.AxisListType.X,
            ALU.add,
        )
        at_ps = psum.tile([128, 128], F32, tag="at_ps", bufs=2)
        nc.tensor.transpose(at_ps[:], abde[:], ident[:])
        abdT = sbuf.tile([128, 128], F32, tag="abdT")
        nc.scalar.copy(abdT[:], at_ps[:])
        nc.vector.tensor_reduce(
            cols["v"][:, tc_col:tc_col + 1], abdT[:], mybir.AxisListType.X,
            ALU.add,
        )
        y_ps = psum.tile([128, 64], F32, tag="y_ps", bufs=2)
        nc.tensor.matmul(y_ps[:], abdT[:], xst[:])
        ysq = sbuf.tile([128, 64], F32, tag="ysq")
        nc.scalar.activation(
            ysq[:], y_ps[:], AF.Square,
            accum_out=cols["rn"][:, tc_col:tc_col + 1],
        )
        y_sb = sbuf.tile([128, 64], F32, tag="y_sb", bufs=46)
        nc.scalar.copy(y_sb[:], y_ps[:])
        cache.append((abde, y_sb))


def _emit_col_math(nc, sbuf, cols, n_all):
    """Batched [128, n_all] column math: d1 and all derived scale vectors."""
    A = slice(0, n_all)
    uu = sbuf.tile([128, n_all], F32, tag="b_uu")
    nc.vector.tensor_scalar(uu[:], cols["u"][:, A], 1e-4, None, ALU.mult)
    rn2 = sbuf.tile([128, n_all], F32, tag="b_rn2")
    nc.vector.tensor_tensor(rn2[:], uu[:], cols["u"][:, A], ALU.mult)
    nc.vector.tensor_tensor(rn2[:], rn2[:], cols["rn"][:, A], ALU.add)
    uv = sbuf.tile([128, n_all], F32, tag="b_uv")
    nc.vector.tensor_tensor(uv[:], cols["u"][:, A], cols["v"][:, A], ALU.mult)
    ruv = sbuf.tile([128, n_all], F32, tag="b_ruv")
    nc.vector.reciprocal(ruv[:], uv[:])
    rat = sbuf.tile([128, n_all], F32, tag="b_rat")
    nc.vector.tensor_tensor(rat[:], rn2[:], ruv[:], ALU.mult)
    nc.scalar.activation(cols["d1"][:, A], rat[:], AF.Sqrt)
    ud1 = sbuf.tile([128, n_all], F32, tag="b_ud1")
    nc.vector.tensor_tensor(ud1[:], cols["u"][:, A], cols["d1"][:, A], ALU.mult)
    nc.vector.reciprocal(cols["sL"][:, A], ud1[:])
    vd1 = sbuf.tile([128, n_all], F32, tag="b_vd1")
    nc.vector.tensor_tensor(vd1[:], cols["v"][:, A], cols["d1"][:, A], ALU.mult)
    nc.vector.reciprocal(cols["sR"][:, A], vd1[:])
    nc.vector.reciprocal(cols["ru"][:, A], cols["u"][:, A])
    nc.vector.reciprocal(cols["rv"][:, A], cols["v"][:, A])
    nc.vector.reciprocal(cols["rd1"][:, A], cols["d1"][:, A])


def _emit_pass3(nc, sbuf, psum, cols, n_units, unit0, ident, cache,
                need_left, dst_L, dst_R):
    """Per-unit: p1/q2 matmuls + operand build.

    Row scaling commutes through the matmul: YtilL = diag(sL) A' X
    = diag(sL) Y, so scale the cached Y and transpose once.
    """
    for t in range(n_units):
        tc_col = unit0 + t
        abde, y_sb = cache[t]
        if need_left:
            p1_ps = psum.tile([128, 1], F32, tag="p1_ps")
            nc.tensor.matmul(p1_ps[:], abde[:],
                             cols["ru"][:, tc_col:tc_col + 1])
            nc.scalar.copy(cols["p1"][:, tc_col:tc_col + 1], p1_ps[:])
            scale = cols["sL"][:, tc_col:tc_col + 1]
            dst = dst_L
        else:
            q2_ps = psum.tile([128, 1], F32, tag="p1_ps")
            nc.tensor.matmul(q2_ps[:], abde[:],
                             cols["rv"][:, tc_col:tc_col + 1])
            nc.scalar.copy(cols["q2c"][:, t:t + 1], q2_ps[:])
            scale = cols["sR"][:, tc_col:tc_col + 1]
            dst = dst_R
        ysc = sbuf.tile([128, 64], F32, tag="ysc")
        nc.vector.tensor_scalar(ysc[:], y_sb[:], scale, None, ALU.mult)
        tl_ps = psum.tile([64, 128], F32, tag="tl_ps", bufs=2)
        nc.tensor.transpose(tl_ps[:], ysc[:], ident[:])
        nc.scalar.copy(dst[0:64, 128 * t:128 * (t + 1)], tl_ps[:])


def build_program():
    nc = bacc.Bacc("TRN2", target_bir_lowering=False, debug=False,
                   enable_asserts=False, num_devices=NCORES)

    A_S = nc.dram_tensor("A_S", [N_S, NN, NN], F32, kind="ExternalInput").ap()
    X_S = nc.dram_tensor("X_S", [N_S, NN, FEAT], F32, kind="ExternalInput").ap()
    y_S = nc.dram_tensor("y_S", [N_S, 10], F32, kind="ExternalInput").ap()
    A_J = nc.dram_tensor("A_J", [NJ_DEV, NN, NN], F32, kind="ExternalInput").ap()
    X_J = nc.dram_tensor("X_J", [NJ_DEV, NN, FEAT], F32, kind="ExternalInput").ap()
    ident_in = nc.dram_tensor("ident", [128, 128], F32, kind="ExternalInput").ap()
    epsident_in = nc.dram_tensor("epsident", [128, 128], F32, kind="ExternalInput").ap()
    ones_in = nc.dram_tensor("ones_row", [1, 128], F32, kind="ExternalInput").ap()
    negident_in = nc.dram_tensor("negident", [128, 128], F32,
                                 kind="ExternalInput").ap()

    pred_out = nc.dram_tensor("pred", [N_T, 10], F32, kind="ExternalOutput").ap()
    kss_out = nc.dram_tensor("K_SS", [N_S, N_S], F32, kind="ExternalOutput").ap()

    with tile.TileContext(nc) as tc:
        with (
            tc.tile_pool(name="persist", bufs=1) as pp,
            tc.tile_pool(name="dram", bufs=1, space="DRAM") as dram,
        ):
            ident = pp.tile([128, 128], F32)
            epsident = pp.tile([128, 128], F32)
            ones_row = pp.tile([1, 128], F32)
            negident = pp.tile([128, 128], F32)
            nc.sync.dma_start(ident[:], ident_in[:])
            nc.sync.dma_start(epsident[:], epsident_in[:])
            nc.sync.dma_start(ones_row[:], ones_in[:])
            nc.sync.dma_start(negident[:], negident_in[:])

            c025 = pp.tile([128, 1], F32)
            nc.vector.memset(c025[:], 0.25)
            c2 = pp.tile([128, 1], F32)
            nc.vector.memset(c2[:], 2.0)
            RHS_L = pp.tile([65, N_S * NN], F32)      # i-side stacked YtilL^T+aug
            LHS_R = pp.tile([65, NJ_DEV * NN], F32)   # j-side stacked YtilR^T+aug
            # per-jblk expanded weight tiles: slice m is [128, 24] with only
            # columns 2m (partitions 0:64) and 2m+1 (partitions 64:128) nonzero
            QD = pp.tile([128, NJ_DEV * N_JBLK], BF16)
            WA = pp.tile([128, NJ_DEV * N_JBLK], BF16)
            WB = pp.tile([128, NJ_DEV * N_JBLK], BF16)
            K_stage = pp.tile([NJ_DEV, N_S], F32)

            cols = {}
            for name in ("u", "v", "rn", "d1", "rd1", "pd", "p", "sL", "sR",
                         "ru", "rv", "p1"):
                cols[name] = pp.tile([128, N_SU + N_JU], F32, name=f"col_{name}")
            for name in ("qdw", "waw", "wbw", "augj", "q2c"):
                cols[name] = pp.tile([128, N_JU], F32, name=f"col_{name}")

            # ---------------- stage A: per-graph precompute ----------------
            with (
                tc.tile_pool(name="pre_sb", bufs=6) as sbuf,
                tc.tile_pool(name="pre_ps", bufs=1, space="PSUM") as psum,
            ):
                cache_S, cache_J = [], []
                _emit_pass1(nc, sbuf, psum, cols, A_S, X_S, N_SU, 0,
                            ident, epsident, cache_S)
                _emit_pass1(nc, sbuf, psum, cols, A_J, X_J, N_JU, N_SU,
                            ident, epsident, cache_J)
                _emit_col_math(nc, sbuf, cols, N_SU + N_JU)
                _emit_pass3(nc, sbuf, psum, cols, N_SU, 0, ident, cache_S,
                            True, RHS_L, None)
                _emit_pass3(nc, sbuf, psum, cols, N_JU, N_SU, ident, cache_J,
                            False, None, LHS_R)
                # batched weight-vector math
                SL, JL = slice(0, N_SU), slice(N_SU, N_SU + N_JU)
                nc.vector.scalar_tensor_tensor(
                    cols["pd"][:, SL], cols["p1"][:, SL], PD_SCALE,
                    cols["d1"][:, SL], ALU.mult, ALU.mult,
                )
                nc.vector.tensor_scalar(
                    cols["p"][:, SL], cols["p1"][:, SL], P_SCALE, None, ALU.mult
                )
                nc.vector.scalar_tensor_tensor(
                    cols["qdw"][:, :], cols["q2c"][:, :], QD_SCALE,
                    cols["d1"][:, JL], ALU.mult, ALU.mult,
                )
                nc.vector.tensor_scalar(
                    cols["waw"][:, :], cols["q2c"][:, :], WA_SCALE, None,
                    ALU.mult,
                )
                nc.vector.tensor_scalar(
                    cols["wbw"][:, :], cols["q2c"][:, :], WB_SCALE, None,
                    ALU.mult,
                )
                nc.vector.scalar_tensor_tensor(
                    cols["augj"][:, :], cols["u"][:, JL], 1e-4,
                    cols["sR"][:, JL], ALU.mult, ALU.mult,
                )

            # aug rows: RHS_L[64, 128*t + suba] = rd1cols[suba, t].
            # Engines can't move data across partitions, so transpose the
            # column tiles on the PE first, then DMA the row-major result.
            with tc.tile_pool(name="augt_ps", bufs=1, space="PSUM") as aps:
                rd1T_ps = aps.tile([N_SU, 128], F32, tag="rd1T_ps")
                nc.tensor.transpose(rd1T_ps[:], cols["rd1"][:, 0:N_SU],
                                    ident[:])
                rd1T = pp.tile([N_SU, 128], F32)
                nc.scalar.copy(rd1T[:], rd1T_ps[:])
                augjT_ps = aps.tile([N_JU, 128], F32, tag="augjT_ps")
                nc.tensor.transpose(augjT_ps[:], cols["augj"][:, 0:N_JU],
                                    ident[:])
                augjT = pp.tile([N_JU, 128], F32)
                nc.scalar.copy(augjT[:], augjT_ps[:])
                pdT_ps = aps.tile([N_SU, 128], F32, tag="rd1T_ps")
                nc.tensor.transpose(pdT_ps[:], cols["pd"][:, 0:N_SU], ident[:])
                pdT = pp.tile([N_SU, 128], F32)
                nc.scalar.copy(pdT[:], pdT_ps[:])
                prT_ps = aps.tile([N_SU, 128], F32, tag="rd1T_ps")
                nc.tensor.transpose(prT_ps[:], cols["p"][:, 0:N_SU], ident[:])
                prT = pp.tile([N_SU, 128], F32)
                nc.scalar.copy(prT[:], prT_ps[:])
            nc.sync.dma_start(
                RHS_L[64:65, :].rearrange("p (t s) -> p t s", s=128), rd1T[:]
            )
            nc.sync.dma_start(
                LHS_R[64:65, :].rearrange("p (t s) -> p t s", s=128), augjT[:]
            )
            # weight block-columns (expanded per jblk for psum-accumulating
            # reduction matmuls): even j -> partitions 0:64 col 24m+2m,
            # odd j -> partitions 64:128 col 24m+2m+1
            for w_all, w_col in ((QD, "qdw"), (WA, "waw"), (WB, "wbw")):
                nc.vector.memset(w_all[:], 0.0)
                for m in range(N_JBLK):
                    base = NJ_DEV * m + 2 * m
                    nc.scalar.copy(w_all[0:64, base:base + 1],
                                   cols[w_col][0:64, m:m + 1])
                    nc.scalar.copy(w_all[64:128, base + 1:base + 2],
                                   cols[w_col][64:128, m:m + 1])

            # ---------------- stage B: pair loop ----------------
            # Engine placement tuned from HW microbenches (ns per [128,512]):
            #   DVE: TT 475, TS 269, STT 602, clip-from-psum 599, recip 3212
            #   GPS: TT ~1279, TS(mult/add dual) 680; min on GPS = 7.4us (!)
            #   ACT: 720/op + ~2.7us per table-set switch (phase-batched)
            # bf16 for phase-crossing storage; fp32 for t1/t2 (cancellation
            # near |S|~1) and arctan outputs.
            act_chain = []

            def act(*args, **kw):
                inst = nc.scalar.activation(*args, **kw)
                act_chain.append(inst)
                return inst

            with (
                tc.tile_pool(name="pb_x", bufs=13) as px,     # phase-crossing
                tc.tile_pool(name="pb_t", bufs=2) as pt,      # in-phase temps
                tc.tile_pool(name="pb_ps", bufs=2, space="PSUM") as pps,
                tc.tile_pool(name="pb_mps", bufs=2, space="PSUM") as mps,
            ):
                for iblk in range(N_IBLK):
                    i0 = iblk * 512
                    m1_ps = mps.tile([NJ_DEV, 512], F32, tag="m1")
                    m2_ps = mps.tile([NJ_DEV, 512], F32, tag="m2")
                    Sc_l, sqh_l, x1_l, t1_l = [], [], [], []
                    P1h_l, S2c_l, sq2_l, x2_l, t2_l = [], [], [], [], []

                    # phase 1: matmul + clip + t1/sqh (sqrt set) + recip + x1
                    for jblk in range(N_JBLK):
                        s1_ps = pps.tile([128, 512], F32, tag="s1")
                        nc.tensor.matmul(
                            s1_ps[:],
                            LHS_R[:, jblk * 128:(jblk + 1) * 128],
                            RHS_L[:, i0:i0 + 512],
                        )
                        Sc = px.tile([128, 512], BF16, tag="Sc")
                        nc.vector.tensor_scalar(
                            Sc[:], s1_ps[:], -A_CLIP, A_CLIP, ALU.max, ALU.min
                        )
                        t1 = pt.tile([128, 512], F32, tag="t1")
                        nc.gpsimd.tensor_tensor(t1[:], Sc[:], Sc[:], ALU.mult)
                        sqh = px.tile([128, 512], BF16, tag="sqh")
                        act(sqh[:], t1[:], AF.Sqrt, bias=c025[:], scale=-0.25)
                        d1s = px.tile([128, 512], BF16, tag="d1s")
                        nc.vector.scalar_tensor_tensor(
                            d1s[:], sqh[:], -4.0, t1[:], ALU.mult, ALU.add
                        )
                        Sc_l.append(Sc); sqh_l.append(sqh); t1_l.append(d1s)

                    for jblk in range(N_JBLK):
                        Sc, sqh, d1s = Sc_l[jblk], sqh_l[jblk], t1_l[jblk]
                        # (1+sq)^2 = 2 - (t1 - 4*sqh); affine folded into ACT
                        rp1 = pt.tile([128, 512], BF16, tag="rp1")
                        act(rp1[:], d1s[:], AF.Abs_reciprocal_sqrt,
                            bias=c2[:], scale=-1.0)
                        x1 = px.tile([128, 512], BF16, tag="x1")
                        nc.gpsimd.tensor_tensor(x1[:], Sc[:], rp1[:], ALU.mult)
                        x1_l.append(x1)

                    # phase 2: arctan L1 (trig set) + P1h + S2p/S2c
                    for jblk in range(N_JBLK):
                        Sc, sqh, x1 = Sc_l[jblk], sqh_l[jblk], x1_l[jblk]
                        at1 = pt.tile([128, 512], F32, tag="at1")
                        act(at1[:], x1[:], AF.Arctan)
                        P1h = px.tile([128, 512], BF16, tag="P1h")
                        nc.vector.scalar_tensor_tensor(
                            P1h[:], at1[:], PI / 4.0, Sc[:], ALU.add, ALU.mult
                        )
                        # S2c is half the reference S2 (constants refolded)
                        S2p = pt.tile([128, 512], BF16, tag="S2p")
                        nc.gpsimd.tensor_tensor(S2p[:], P1h[:], sqh[:], ALU.add)
                        S2c = px.tile([128, 512], BF16, tag="S2c")
                        nc.vector.tensor_scalar(S2c[:], S2p[:], 1.0,
                                                CLIP2 / 2.0, ALU.mult, ALU.min)
                        P1h_l.append(P1h); S2c_l.append(S2c)

                    # phase 3: t2/sq2 (sqrt set) + recip + x2
                    for jblk in range(N_JBLK):
                        S2c = S2c_l[jblk]
                        t2 = pt.tile([128, 512], F32, tag="t2")
                        nc.gpsimd.tensor_tensor(t2[:], S2c[:], S2c[:], ALU.mult)
                        sq2 = px.tile([128, 512], BF16, tag="sq2")
                        act(sq2[:], t2[:], AF.Sqrt,
                            bias=1.0, scale=-(4.0 * K_CONST * K_CONST))
                        d2s = px.tile([128, 512], BF16, tag="d2s")
                        nc.vector.scalar_tensor_tensor(
                            d2s[:], sq2[:], -2.0 / (4.0 * K_CONST * K_CONST),
                            t2[:], ALU.mult, ALU.add,
                        )
                        sq2_l.append(sq2); t2_l.append(d2s)

                    for jblk in range(N_JBLK):
                        S2c, sq2, d2s = S2c_l[jblk], sq2_l[jblk], t2_l[jblk]
                        # (1+sq2)^2 = 2 - 4K^2*(t2 - sq2/(2K^2))
                        rp2 = pt.tile([128, 512], BF16, tag="rp2")
                        act(rp2[:], d2s[:], AF.Abs_reciprocal_sqrt,
                            bias=c2[:], scale=-(4.0 * K_CONST * K_CONST))
                        x2 = px.tile([128, 512], BF16, tag="x2")
                        nc.vector.scalar_tensor_tensor(
                            x2[:], S2c[:], 2.0 * K_CONST, rp2[:], ALU.mult,
                            ALU.mult,
                        )
                        x2_l.append(x2)

                    # phase 4: arctan L2 (trig set) + G1h/m2h + reduction MMs
                    for jblk in range(N_JBLK):
                        P1h, S2c, sq2, x2 = (P1h_l[jblk], S2c_l[jblk],
                                             sq2_l[jblk], x2_l[jblk])
                        at2 = pt.tile([128, 512], F32, tag="at2")
                        act(at2[:], x2[:], AF.Arctan)
                        G1h = pt.tile([128, 512], BF16, tag="G1h")
                        nc.vector.scalar_tensor_tensor(
                            G1h[:], at2[:], PI / 4.0, P1h[:], ALU.add, ALU.mult
                        )
                        m2h = pt.tile([128, 512], BF16, tag="m2h")
                        nc.vector.scalar_tensor_tensor(
                            m2h[:], at2[:], PI / 4.0, S2c[:], ALU.add, ALU.mult
                        )
                        wslice = slice(NJ_DEV * jblk, NJ_DEV * (jblk + 1))
                        nc.tensor.matmul(
                            m1_ps[:], QD[:, wslice], G1h[:],
                            start=(jblk == 0), stop=(jblk == N_JBLK - 1),
                        )
                        nc.tensor.matmul(
                            m2_ps[:], WA[:, wslice], m2h[:],
                            start=(jblk == 0), stop=False,
                        )
                        nc.tensor.matmul(
                            m2_ps[:], WB[:, wslice], sq2[:],
                            start=False, stop=(jblk == N_JBLK - 1),
                        )

                    # finalize iblk: weighted a-reduction -> K_stage columns
                    pdf = pt.tile([1, 512], F32, tag="pdf")
                    prf = pt.tile([1, 512], F32, tag="prf")
                    nc.sync.dma_start(
                        pdf[:].rearrange("p (t s) -> p t s", s=128),
                        pdT[iblk * 4:(iblk + 1) * 4, :],
                    )
                    nc.sync.dma_start(
                        prf[:].rearrange("p (t s) -> p t s", s=128),
                        prT[iblk * 4:(iblk + 1) * 4, :],
                    )
                    pd_ps = pps.tile([NJ_DEV, 512], F32, tag="pdb_ps")
                    nc.tensor.matmul(pd_ps[:], ones_row[:, 0:NJ_DEV], pdf[:])
                    pdb = pt.tile([NJ_DEV, 512], F32, tag="pdb")
                    nc.scalar.copy(pdb[:], pd_ps[:])
                    pr_ps = pps.tile([NJ_DEV, 512], F32, tag="pdb_ps")
                    nc.tensor.matmul(pr_ps[:], ones_row[:, 0:NJ_DEV], prf[:])
                    prb = pt.tile([NJ_DEV, 512], F32, tag="prb")
                    nc.scalar.copy(prb[:], pr_ps[:])

                    w1 = pt.tile([NJ_DEV, 512], F32, tag="w1")
                    nc.vector.tensor_tensor(w1[:], m1_ps[:], pdb[:], ALU.mult)
                    w2 = pt.tile([NJ_DEV, 512], F32, tag="w2")
                    nc.vector.tensor_tensor(w2[:], m2_ps[:], prb[:], ALU.mult)
                    r1 = pt.tile([NJ_DEV, 8], F32, tag="r1")
                    nc.vector.tensor_reduce(
                        r1[:], w1[:].rearrange("p (j a) -> p j a", j=8),
                        mybir.AxisListType.X, ALU.add,
                    )
                    r2 = pt.tile([NJ_DEV, 8], F32, tag="r2")
                    nc.vector.tensor_reduce(
                        r2[:], w2[:].rearrange("p (j a) -> p j a", j=8),
                        mybir.AxisListType.X, ALU.add,
                    )
                    nc.vector.tensor_tensor(
                        K_stage[:, iblk * 8:(iblk + 1) * 8], r1[:], r2[:], ALU.add
                    )

            # serialize ACT transcendentals in emission order so the scheduler
            # cannot interleave table sets (each switch costs ~2.7us)
            from concourse.tile_rust import add_dep_helper
            for a, b in zip(act_chain[1:], act_chain[:-1]):
                add_dep_helper(a.ins, b.ins, reason="act table-set phase order")

            # ---------------- stage C: all-gather ----------------
            cc_in = dram.tile([NJ_DEV, N_S], F32)
            cc_out = dram.tile([NJ, N_S], F32, addr_space="Shared")
            nc.sync.dma_start(cc_in[:], K_stage[:])
            nc.gpsimd.collective_compute(
                "AllGather",
                ALU.bypass,
                ins=[cc_in[:].opt()],
                outs=[cc_out[:].opt()],
                replica_groups=[list(range(NCORES))],
            )

            # ---------------- stage D: solve + outputs ----------------
            with (
                tc.tile_pool(name="sol_sb", bufs=2) as ss,
                tc.tile_pool(name="sol_ps", bufs=1, space="PSUM") as sps,
            ):
                kssT = ss.tile([64, 64], F32, tag="kssT")
                nc.sync.dma_start(kssT[:], cc_out[0:64, :])
                kstT = ss.tile([128, 64], F32, tag="kstT")
                nc.sync.dma_start(kstT[:], cc_out[64:NJ, :])
                y_sb = ss.tile([64, 10], F32, tag="ysb")
                nc.sync.dma_start(y_sb[:], y_S[:])

                kss_ps = sps.tile([64, 64], F32, tag="kss_ps")
                nc.tensor.transpose(kss_ps[:], kssT[:], ident[0:64, 0:64])
                kss_sb = ss.tile([64, 64], F32, tag="kss_sb")
                nc.scalar.copy(kss_sb[:], kss_ps[:])
                nc.sync.dma_start(kss_out[:], kss_sb[:])

                # lambda = 1e-6 * tr(K_SS)/64, broadcast to [64,1]
                dd = ss.tile([64, 64], F32, tag="dd")
                nc.vector.tensor_tensor(dd[:], kss_sb[:], ident[0:64, 0:64],
                                        ALU.mult)
                dcol = ss.tile([64, 1], F32, tag="dcol")
                nc.vector.tensor_reduce(dcol[:], dd[:], mybir.AxisListType.X,
                                        ALU.add)
                from concourse import bass_isa
                tr_all = ss.tile([64, 1], F32, tag="tr_all")
                nc.gpsimd.partition_all_reduce(tr_all[:], dcol[:], 64,
                                               bass_isa.ReduceOp.add)
                lamcol = ss.tile([64, 1], F32, tag="lamcol")
                nc.vector.tensor_scalar(lamcol[:], tr_all[:], 1e-6 / 64.0,
                                        None, ALU.mult)

                aug = ss.tile([64, 74], F32, tag="aug")
                nc.vector.scalar_tensor_tensor(
                    aug[:, 0:64], ident[0:64, 0:64], lamcol[:], kss_sb[:],
                    ALU.mult, ALU.add,
                )
                nc.scalar.copy(aug[:, 64:74], y_sb[:])

                for k in range(64):
                    # broadcast row k of aug via a zero-stride one-hot lhsT:
                    # out[m,n] = sum_p ident[p,k] * aug[p,n] = aug[k,n]
                    row_ps = sps.tile([64, 74], F32, tag="row_ps", bufs=2)
                    nc.tensor.matmul(
                        row_ps[:],
                        ident[0:64, k:k + 1].broadcast_to([64, 64]),
                        aug[:, :],
                    )
                    rpiv = ss.tile([64, 1], F32, tag="rpiv")
                    nc.vector.reciprocal(rpiv[:], row_ps[:, k:k + 1])
                    # nf = -aug[:,k]/piv with nf[k]=0, via negident col
                    # (negident[p,k] = -1 if p!=k else 0)
                    nf = ss.tile([64, 1], F32, tag="nf")
                    nc.vector.scalar_tensor_tensor(
                        nf[:], aug[:, k:k + 1], rpiv[:],
                        negident[0:64, k:k + 1], ALU.mult, ALU.mult,
                    )
                    aug_n = ss.tile([64, 74], F32, tag="aug")
                    nc.vector.scalar_tensor_tensor(
                        aug_n[:], row_ps[:], nf[:], aug[:], ALU.mult, ALU.add
                    )
                    aug = aug_n

                dd2 = ss.tile([64, 64], F32, tag="dd")
                nc.vector.tensor_tensor(dd2[:], aug[:, 0:64], ident[0:64, 0:64],
                                        ALU.mult)
                dcol2 = ss.tile([64, 1], F32, tag="dcol")
                nc.vector.tensor_reduce(dcol2[:], dd2[:], mybir.AxisListType.X,
                                        ALU.add)
                rdg = ss.tile([64, 1], F32, tag="rdg")
                nc.vector.reciprocal(rdg[:], dcol2[:])
                Z = ss.tile([64, 10], F32, tag="Z")
                nc.vector.tensor_scalar(Z[:], aug[:, 64:74], rdg[:], None,
                                        ALU.mult)

                kst_ps = sps.tile([64, 128], F32, tag="kst_ps")
                nc.tensor.transpose(kst_ps[:], kstT[:], ident[:])
                kst_sb = ss.tile([64, 128], F32, tag="kst_sb")
                nc.scalar.copy(kst_sb[:], kst_ps[:])
                pred_ps = sps.tile([128, 10], F32, tag="pred_ps")
                nc.tensor.matmul(pred_ps[:], kst_sb[:], Z[:])
                pred_sb = ss.tile([128, 10], F32, tag="pred_sb")
                nc.scalar.copy(pred_sb[:], pred_ps[:])
                nc.sync.dma_start(pred_out[:], pred_sb[:])

    nc.compile()
    return nc


_PROGRAM = None


def _get_program():
    global _PROGRAM
    if _PROGRAM is None:
        _PROGRAM = build_program()
    return _PROGRAM


def make_in_maps(A_S, X_S, y_S, A_T, X_T):
    A_S = np.ascontiguousarray(A_S, dtype=np.float32)
    X_S = np.ascontiguousarray(X_S, dtype=np.float32)
    y_S = np.ascontiguousarray(y_S, dtype=np.float32)
    A_T = np.ascontiguousarray(A_T, dtype=np.float32)
    X_T = np.ascontiguousarray(X_T, dtype=np.float32)
    A_all = np.concatenate([A_S, A_T], axis=0)
    X_all = np.concatenate([X_S, X_T], axis=0)
    ident = np.eye(128, dtype=np.float32)
    epsident = (1e-4 * np.eye(128)).astype(np.float32)
    ones_row = np.ones((1, 128), dtype=np.float32)
    in_maps = []
    for d in range(NCORES):
        in_maps.append({
            "A_S": A_S, "X_S": X_S, "y_S": y_S,
            "A_J": A_all[d * NJ_DEV:(d + 1) * NJ_DEV],
            "X_J": X_all[d * NJ_DEV:(d + 1) * NJ_DEV],
            "ident": ident, "epsident": epsident, "ones_row": ones_row,
            "negident": (np.eye(128, dtype=np.float32) - 1.0),
        })
    return in_maps


def kernel(A_S, X_S, y_S, A_T, X_T):
    nc = _get_program()
    in_maps = make_in_maps(A_S, X_S, y_S, A_T, X_T)
    res = bass_utils.run_bass_kernel_spmd(
        nc, in_maps, core_ids=list(range(NCORES))
    )
    pred = np.asarray(res.results[0]["pred"], dtype=np.float32)
    kss = np.asarray(res.results[0]["K_SS"], dtype=np.float32)
    return pred, kss
